# revision 1
# baseline (speedup 1.0000x reference)
"""DCRNN (nn_DCRNNModel) forward pass on 8 Trainium2 NeuronCores.

Strategy: data-parallel over batch (B=32 -> 4 chains/core). All weights and
both random-walk supports are replicated per core and live in SBUF for the
whole kernel. The DCGRU cell is restructured into the diffusion "power basis"
    gconv(z) = z*W0' + (S0 z)W1 + (S0^2 z)(2 W2) + (S1 z)W3 + (S1^2 z)(2 W4)
with W0' = W0 - W2 - W4, so each projection is a plain K<=128 matmul.

Layouts (per chain):
  feature-major tiles [feat, node]: h-features on partitions 0:64,
    x-features on partitions 64:64+I (everything elementwise runs at base 0).
  node-major tiles zn [128, 8, 128]: partition=node%128, kt=node//128,
    free cols 0:64 = h-part, 64:64+I = x-part (zero padded).
Diffusion computes V1T = (S z)^T feature-major via lhsT=zn tiles, rhs=S^T
tiles; V1 is re-transposed to node-major with an identity matmul to build
V2T = (S V1)^T. Projections consume the feature-major tiles directly.
All matmuls run in float32r (full-rate fp32 on the PE).
"""
import sys
import os
import time as _time
import numpy as np

sys.path.insert(0, '/opt/trn_rl_repo')

N = 1024
B = 32
T_FULL = 12
HOR_FULL = 12
HID = 64
N_CORES = 8
CHAINS = 4           # batch elements per core
KT = 8               # node k-tiles (1024/128)
P = 128

_cache = {}


def _build(T, HOR):
    import concourse.bacc as bacc
    import concourse.tile as tile
    from concourse import mybir

    F32R = mybir.dt.float32r
    F32 = mybir.dt.float32
    AF = mybir.ActivationFunctionType

    _t0 = _time.time()
    nc = bacc.Bacc('TRN2', target_bir_lowering=False, debug=False,
                   num_devices=N_CORES)

    LAYERS = (('enc0', 2), ('enc1', 64), ('dec0', 1), ('dec1', 64))

    # ---- DRAM declarations ----
    d = {}
    d['s0t'] = nc.dram_tensor('s0t', [P, KT, N], F32R, kind='ExternalInput').ap()
    d['s1t'] = nc.dram_tensor('s1t', [P, KT, N], F32R, kind='ExternalInput').ap()
    d['ident'] = nc.dram_tensor('ident', [P, P], F32R, kind='ExternalInput').ap()
    d['idhi'] = nc.dram_tensor('idhi', [P, HID], F32R, kind='ExternalInput').ap()
    d['xT'] = nc.dram_tensor('xT', [CHAINS, T, 2, N], F32R, kind='ExternalInput').ap()
    d['xn'] = nc.dram_tensor('xn', [CHAINS, T, P, KT, 2], F32R, kind='ExternalInput').ap()
    for L, I in LAYERS:
        d[L + '_wa'] = nc.dram_tensor(L + '_wa', [P, 5, P], F32R, kind='ExternalInput').ap()
        d[L + '_wc'] = nc.dram_tensor(L + '_wc', [P, 5, HID], F32R, kind='ExternalInput').ap()
        d[L + '_wc0h'] = nc.dram_tensor(L + '_wc0h', [HID, HID], F32R, kind='ExternalInput').ap()
        d[L + '_bru'] = nc.dram_tensor(L + '_bru', [P, 1], F32, kind='ExternalInput').ap()
        d[L + '_bc'] = nc.dram_tensor(L + '_bc', [HID, 1], F32, kind='ExternalInput').ap()
        if I == 64:
            d[L + '_wa0x'] = nc.dram_tensor(L + '_wa0x', [HID, P], F32R, kind='ExternalInput').ap()
            d[L + '_wc0x'] = nc.dram_tensor(L + '_wc0x', [HID, HID], F32R, kind='ExternalInput').ap()
    d['dec0_wa0x1'] = nc.dram_tensor('dec0_wa0x1', [P, P], F32R, kind='ExternalInput').ap()
    d['dec0_wc0x1'] = nc.dram_tensor('dec0_wc0x1', [P, HID], F32R, kind='ExternalInput').ap()
    d['fcnw'] = nc.dram_tensor('fcnw', [HID, 1], F32R, kind='ExternalInput').ap()
    d['fcnw8'] = nc.dram_tensor('fcnw8', [HID, 8], F32R, kind='ExternalInput').ap()
    d['fcnb'] = nc.dram_tensor('fcnb', [P, 1], F32, kind='ExternalInput').ap()
    d['zeros'] = nc.dram_tensor('zeros', [P, N], F32R, kind='ExternalInput').ap()
    d_out = nc.dram_tensor('out', [HOR, CHAINS, N], F32, kind='ExternalOutput').ap()

    _temit = _time.time()
    with tile.TileContext(nc) as tc:
        with tc.tile_pool(name='const', bufs=1) as const, \
             tc.tile_pool(name='state', bufs=1) as state, \
             tc.tile_pool(name='vt', bufs=4) as vtp, \
             tc.tile_pool(name='vn', bufs=2) as vnp, \
             tc.tile_pool(name='ru', bufs=1) as rup, \
             tc.tile_pool(name='u0', bufs=1) as u0p, \
             tc.tile_pool(name='rh', bufs=2) as rhp, \
             tc.tile_pool(name='ct', bufs=1) as ctp, \
             tc.tile_pool(name='tg', bufs=1) as tgp, \
             tc.tile_pool(name='yt', bufs=1) as ytp, \
             tc.tile_pool(name='pd', bufs=2, space='PSUM') as pdp, \
             tc.tile_pool(name='pp', bufs=1, space='PSUM') as ppp, \
             tc.tile_pool(name='pt', bufs=1, space='PSUM') as ptp:

            # ---- load constants ----
            cst = {}
            for name, dd in d.items():
                if name in ('xT', 'xn', 'zeros'):
                    continue
                shape = list(dd.shape)
                dt_ = dd.dtype
                t = const.tile(shape, dt_, tag=name, name='cst_' + name)
                nc.sync.dma_start(t[:], dd[:])
                cst[name] = t
            s0t, s1t = cst['s0t'], cst['s1t']
            ident, idhi = cst['ident'], cst['idhi']

            # ---- persistent state ----
            d_zeros = d['zeros']
            zn_zeros = d_zeros.rearrange('p (kt f) -> p kt f', kt=KT)
            zT0, zT1, zn0, zn1 = [], [], [], []
            yall = state.tile([P, N], F32R, tag='yall', name='yall')
            nc.sync.dma_start(yall[:], d_zeros[:])
            for c in range(CHAINS):
                zT0.append(state.tile([P, N], F32R, tag=f'zT0_{c}', name=f'zT0_{c}'))
                zT1.append(state.tile([HID, N], F32R, tag=f'zT1_{c}', name=f'zT1_{c}'))
                zn0.append(state.tile([P, KT, P], F32R, tag=f'zn0_{c}', name=f'zn0_{c}'))
                zn1.append(state.tile([P, KT, P], F32R, tag=f'zn1_{c}', name=f'zn1_{c}'))
                nc.sync.dma_start(zT0[c][:], d_zeros[:])
                nc.sync.dma_start(zT1[c][:], d_zeros[0:HID, :])
                nc.sync.dma_start(zn0[c][:], zn_zeros[:])
                nc.sync.dma_start(zn1[c][:], zn_zeros[:])

            def diffuse(zn_tile, stt):
                """V = (S z)^T feature-major [128, N] (f32r sbuf tile)."""
                p = pdp.tile([P, N], F32, tag='pd')
                for ms in range(2):
                    sl = slice(ms * 512, (ms + 1) * 512)
                    for kt in range(KT):
                        nc.tensor.matmul(p[:, sl], zn_tile[:, kt, :],
                                         stt[:, kt, sl],
                                         start=(kt == 0), stop=(kt == KT - 1))
                v = vtp.tile([P, N], F32R, tag='vt')
                nc.vector.tensor_copy(v[:], p[:])
                return v

            def to_node(vt_tile):
                """feature-major [128, N] -> node-major [128, KT, 128]."""
                p = ptp.tile([P, KT, P], F32, tag='pt')
                for m in range(KT):
                    nc.tensor.matmul(p[:, m, :], vt_tile[:, m * P:(m + 1) * P],
                                     ident[:], start=True, stop=True)
                v = vnp.tile([P, KT, P], F32R, tag='vn')
                nc.vector.tensor_copy(v[:], p[:])
                return v

            def gconv_vtiles(zn_tile):
                vts = []
                for stt in (s0t, s1t):
                    v1 = diffuse(zn_tile, stt)
                    v1n = to_node(v1)
                    v2 = diffuse(v1n, stt)
                    vts += [v1, v2]
                return vts

            def cell(L, I, c, is_layer0, t_idx, phase):
                znL = zn0[c] if is_layer0 else zn1[c]
                zTh = zT0[c] if is_layer0 else zT1[c]   # h rows 0:64
                wa = cst[L + '_wa']
                wc = cst[L + '_wc']

                # --- gA ---
                vts = gconv_vtiles(znL)
                pa = ppp.tile([P, N], F32, tag='pp')
                for ms in range(2):
                    sl = slice(ms * 512, (ms + 1) * 512)
                    mms = []
                    # identity terms
                    if L == 'enc0':
                        mms.append((wa[:, 0, :], zT0[c][:, sl], None))
                    elif L == 'dec0':
                        mms.append((wa[:, 0, :], zT0[c][:, sl], None))
                        b32 = 32 * c
                        mms.append((cst['dec0_wa0x1'][b32:b32 + 1, :],
                                    yall[b32:b32 + 1, sl], (b32, 0)))
                    else:
                        mms.append((wa[0:HID, 0, :], zTh[0:HID, sl], None))
                        mms.append((cst[L + '_wa0x'][:], zT0[c][0:HID, sl], None))
                    for i in range(4):
                        mms.append((wa[:, i + 1, :], vts[i][:, sl], None))
                    for j, (lhs, rhs, tp) in enumerate(mms):
                        nc.tensor.matmul(pa[:, sl], lhs, rhs,
                                         start=(j == 0), stop=(j == len(mms) - 1),
                                         tile_position=tp)
                ru = rup.tile([P, N], F32R, tag='ru')
                nc.scalar.activation(ru[:], pa[:], AF.Sigmoid,
                                     bias=cst[L + '_bru'][:])
                # u = ru rows 64:128 shifted down to base 0 via idhi
                psh = ptp.tile([P, N], F32, tag='pt')
                for ms in range(2):
                    sl = slice(ms * 512, (ms + 1) * 512)
                    nc.tensor.matmul(psh[0:HID, sl], idhi[:], ru[:, sl],
                                     start=True, stop=True)
                u0 = u0p.tile([HID, N], F32, tag='u0')
                nc.vector.tensor_copy(u0[:], psh[0:HID, :])
                # rh = r * h
                rh = rhp.tile([HID, N], F32R, tag='rh')
                nc.vector.tensor_mul(rh[:], ru[0:HID, :], zTh[0:HID, :])
                # rh -> node (overwrite h-slot of znL)
                prh = ptp.tile([P, KT, P], F32, tag='pt')
                for m in range(KT):
                    nc.tensor.matmul(prh[:, m, 0:HID], rh[:, m * P:(m + 1) * P],
                                     ident[0:HID, 0:HID], start=True, stop=True)
                nc.vector.tensor_copy(znL[:, :, 0:HID], prh[:, :, 0:HID])

                # --- gB ---
                vts2 = gconv_vtiles(znL)
                pb = ppp.tile([P, N], F32, tag='pp')
                for ms in range(2):
                    sl = slice(ms * 512, (ms + 1) * 512)
                    mms = [(cst[L + '_wc0h'][:], rh[:, sl], None)]
                    if L == 'enc0':
                        mms.append((wc[:, 0, :], zT0[c][:, sl], None))
                    elif L == 'dec0':
                        b32 = 32 * c
                        mms.append((cst['dec0_wc0x1'][b32:b32 + 1, :],
                                    yall[b32:b32 + 1, sl], (b32, 0)))
                    else:
                        mms.append((cst[L + '_wc0x'][:], zT0[c][0:HID, sl], None))
                    for i in range(4):
                        mms.append((wc[:, i + 1, :], vts2[i][:, sl], None))
                    for j, (lhs, rhs, tp) in enumerate(mms):
                        nc.tensor.matmul(pb[0:HID, sl], lhs, rhs,
                                         start=(j == 0), stop=(j == len(mms) - 1),
                                         tile_position=tp)
                ct = ctp.tile([HID, N], F32, tag='ct')
                nc.scalar.activation(ct[:], pb[0:HID, :], AF.Tanh,
                                     bias=cst[L + '_bc'][:])
                # --- GRU: h' = c + u*(h-c) ---
                t1 = tgp.tile([HID, N], F32, tag='tg')
                nc.vector.tensor_sub(t1[:], zTh[0:HID, :], ct[:])
                nc.vector.tensor_mul(t1[:], t1[:], u0[:])
                nc.vector.tensor_add(zTh[0:HID, :], ct[:], t1[:])
                # --- h' -> node ---
                ph = ptp.tile([P, KT, P], F32, tag='pt')
                for m in range(KT):
                    nc.tensor.matmul(ph[:, m, 0:HID], zTh[0:HID, m * P:(m + 1) * P],
                                     ident[0:HID, 0:HID], start=True, stop=True)
                nc.vector.tensor_copy(znL[:, :, 0:HID], ph[:, :, 0:HID])
                if is_layer0:
                    nc.vector.tensor_copy(zn1[c][:, :, HID:P], ph[:, :, 0:HID])

            # ================= encoder =================
            for t in range(T):
                for c in range(CHAINS):
                    nc.sync.dma_start(zT0[c][HID:HID + 2, :], d['xT'][c, t])
                    nc.sync.dma_start(zn0[c][:, :, HID:HID + 2], d['xn'][c, t])
                    cell('enc0', 2, c, True, t, 'enc')
                    cell('enc1', 64, c, False, t, 'enc')

            # ================= decoder =================
            for c in range(CHAINS):
                nc.sync.dma_start(zT0[c][HID:HID + 2, :], d_zeros[0:2, :])
                nc.sync.dma_start(zn0[c][:, :, HID:HID + 2],
                                  d_zeros[:, 0:2 * KT].rearrange('p (kt f) -> p kt f', kt=KT))
            nc.sync.dma_start(yall[:], d_zeros[:])
            for t in range(HOR):
                for c in range(CHAINS):
                    cell('dec0', 1, c, True, t, 'dec')
                    cell('dec1', 64, c, False, t, 'dec')
                    # fcn: yT and y-node from h1' (zT1 rows 0:64)
                    py = ptp.tile([P, N], F32, tag='pt')
                    for ms in range(2):
                        sl = slice(ms * 512, (ms + 1) * 512)
                        nc.tensor.matmul(py[0:1, sl], cst['fcnw'][:],
                                         zT1[c][0:HID, sl], start=True, stop=True)
                    ytmp = ytp.tile([1, N], F32R, tag='yt', name='ytmp')
                    nc.scalar.activation(ytmp[:], py[0:1, :], AF.Identity,
                                         bias=cst['fcnb'][0:1, :])
                    nc.sync.dma_start(d_out[t, c:c + 1, :], ytmp[:].bitcast(F32))
                    nc.sync.dma_start(yall[32 * c:32 * c + 1, :], ytmp[:])
                    pyn = ptp.tile([P, KT, P], F32, tag='pt')
                    for m in range(KT):
                        nc.tensor.matmul(pyn[:, m, 0:8], zT1[c][0:HID, m * P:(m + 1) * P],
                                         cst['fcnw8'][:], start=True, stop=True)
                    nc.scalar.activation(zn0[c][:, :, HID:HID + 1], pyn[:, :, 0:1],
                                         AF.Identity, bias=cst['fcnb'][:])

    print(f'[build] emission+schedule: {_time.time() - _t0:.1f}s', flush=True)
    _t1 = _time.time()
    nc.compile()
    print(f'[build] bacc compile: {_time.time() - _t1:.1f}s', flush=True)
    return nc


def _prep_host(inputs):
    """Host-side preprocessing -> per-core in_maps."""
    f32 = np.float32
    adj = np.asarray(inputs['adj'], f32)
    source = np.asarray(inputs['source'], f32)       # [B, T, N, 2]

    def rw(a):
        return (a / np.maximum(a.sum(1, keepdims=True), np.float32(1e-8))).astype(f32)

    s0t = rw(adj)            # rhs tiles: ST[n, m] with S0 = rw(adj).T
    s1t = rw(adj.T)

    def tile_nm(a):          # [N, N] -> [128, KT, N]
        return np.ascontiguousarray(a.reshape(KT, P, N).transpose(1, 0, 2))

    common = {
        's0t': tile_nm(s0t),
        's1t': tile_nm(s1t),
        'ident': np.eye(P, dtype=f32),
    }
    idhi = np.zeros((P, HID), f32)
    idhi[HID:P] = np.eye(HID, dtype=f32)
    common['idhi'] = idhi

    for L, I in (('enc0', 2), ('enc1', 64), ('dec0', 1), ('dec1', 64)):
        F = I + HID
        Wru = np.asarray(inputs[L + '_Wru'], f32)
        Wc = np.asarray(inputs[L + '_Wc'], f32)
        bru = np.asarray(inputs[L + '_bru'], f32)
        bc = np.asarray(inputs[L + '_bc'], f32)
        bA = [Wru[i * F:(i + 1) * F] for i in range(5)]
        bC = [Wc[i * F:(i + 1) * F] for i in range(5)]
        eA = [bA[0] - bA[2] - bA[4], bA[1], 2 * bA[2], bA[3], 2 * bA[4]]
        eC = [bC[0] - bC[2] - bC[4], bC[1], 2 * bC[2], bC[3], 2 * bC[4]]
        wa = np.zeros((P, 5, P), f32)
        wc = np.zeros((P, 5, HID), f32)
        for i in range(5):
            wa[0:HID, i] = eA[i][I:F]        # h-part rows 0:64
            wa[HID:HID + I, i] = eA[i][0:I]  # x-part rows 64:64+I
            if i > 0:
                wc[0:HID, i] = eC[i][I:F]
                wc[HID:HID + I, i] = eC[i][0:I]
        # gB identity: h-part via rh tile (wc0h), x-part per layer
        common[L + '_wc0h'] = np.ascontiguousarray(eC[0][I:F])
        if L == 'enc0':
            wc[HID:HID + I, 0] = eC[0][0:I]  # x-part read from zT0 (K=128)
        elif L == 'dec0':
            wa0x1 = np.zeros((P, P), f32)
            wc0x1 = np.zeros((P, HID), f32)
            for cc in range(4):
                wa0x1[32 * cc] = eA[0][0]
                wc0x1[32 * cc] = eC[0][0]
            common['dec0_wa0x1'] = wa0x1
            common['dec0_wc0x1'] = wc0x1
            wa[HID:HID + I, 0] = 0           # y handled via K=1 terms
        else:
            common[L + '_wa0x'] = np.ascontiguousarray(eA[0][0:I])
            common[L + '_wc0x'] = np.ascontiguousarray(eC[0][0:I])
        common[L + '_wa'] = wa
        common[L + '_wc'] = wc
        common[L + '_bru'] = bru.reshape(P, 1).copy()
        common[L + '_bc'] = bc.reshape(HID, 1).copy()

    common['zeros'] = np.zeros((P, N), f32)
    common['fcnw'] = np.asarray(inputs['fcn_W'], f32).reshape(HID, 1).copy()
    common['fcnw8'] = np.repeat(common['fcnw'], 8, axis=1).copy()
    common['fcnb'] = np.full((P, 1), np.asarray(inputs['fcn_b'], f32).reshape(-1)[0], f32)

    T = _cache['T']
    in_maps = []
    for core in range(N_CORES):
        m = dict(common)
        xT = np.zeros((CHAINS, T, 2, N), f32)
        xn = np.zeros((CHAINS, T, P, KT, 2), f32)
        for c in range(CHAINS):
            b = core * CHAINS + c
            for t in range(T):
                xt = source[b, t]            # [N, 2]
                xT[c, t] = xt.T
                xn[c, t] = xt.reshape(KT, P, 2).transpose(1, 0, 2)
        m['xT'] = xT
        m['xn'] = xn
        in_maps.append(m)
    return in_maps


def kernel(**inputs):
    from concourse import bass_utils

    T = int(os.environ.get('DCRNN_T', T_FULL))
    HOR = int(os.environ.get('DCRNN_HOR', HOR_FULL))
    key = (T, HOR)
    if _cache.get('key') != key:
        _cache['nc'] = _build(T, HOR)
        _cache['key'] = key
        _cache['T'] = T
        _cache['HOR'] = HOR
    _cache['T'] = T

    in_maps = _prep_host(inputs)
    res = bass_utils.run_bass_kernel_spmd(
        _cache['nc'], in_maps, core_ids=list(range(N_CORES)),
        trace=bool(int(os.environ.get('DCRNN_TRACE', '0'))))
    _cache['last_res'] = res

    HORr = _cache['HOR']
    out = np.zeros((HORr, B, N), np.float32)
    for core in range(N_CORES):
        r = res.results[core]['out']         # [HOR, CHAINS, N]
        for c in range(CHAINS):
            out[:, core * CHAINS + c, :] = r[:, c, :]
    return out



# revision 4
# speedup vs baseline: 2.1001x; 2.1001x over previous
"""DCRNN (nn_DCRNNModel) forward pass on 8 Trainium2 NeuronCores.

Data-parallel over batch (B=32 -> 4 chains/core). All weights plus the four
diffusion operators S0, S0^2, S1, S1^2 (S^2 precomputed on host, so the
Chebyshev second power needs no chained matmul / transpose) live in SBUF in
fp16 for the whole kernel. All PE matmuls run in fp16 (1 cycle/row at any
moving size); PSUM accumulates fp32.

Per DCGRU cell:
  gate A diffuses the per-chain node-major pack zn=[h|x] (4 matmul sets);
  gate B diffuses only r*h, pair-packed two chains per 128-wide pack
  (halving gate-B diffusion); gate-B x-terms reuse gate A's V tiles through
  zero-padded K=128 weights so every operand stays at partition base 0
  (HW requires equal base partitions and stable PE tile positions inside an
  accumulation group).
Elementwise GRU math runs per-chain at base 0 in fp16 (sigmoid/tanh are
split into 64-partition halves so r, u, c all land at base 0).
"""
import sys
import os
import time as _time
import numpy as np

sys.path.insert(0, '/opt/trn_rl_repo')

N = 1024
B = 32
T_FULL = 12
HOR_FULL = 12
HID = 64
N_CORES = 8
CHAINS = 4
KT = 8
P = 128

_cache = {}

LAYERS = (('enc0', 2), ('enc1', 64), ('dec0', 1), ('dec1', 64))


def _build(T, HOR):
    import concourse.bacc as bacc
    import concourse.tile as tile
    from concourse import mybir

    F32 = mybir.dt.float32
    F16 = mybir.dt.float16
    AF = mybir.ActivationFunctionType

    _t0 = _time.time()
    nc = bacc.Bacc('TRN2', target_bir_lowering=False, debug=False,
                   num_devices=N_CORES)

    # ---- DRAM declarations ----
    d = {}
    for m in range(4):
        d[f's_t{m}'] = nc.dram_tensor(f's_t{m}', [P, KT, N], F16,
                                      kind='ExternalInput').ap()
    d['ident'] = nc.dram_tensor('ident', [P, P], F16, kind='ExternalInput').ap()
    d['xT'] = nc.dram_tensor('xT', [CHAINS, T, 2, N], F16,
                             kind='ExternalInput').ap()
    d['xn'] = nc.dram_tensor('xn', [CHAINS, T, P, KT, 2], F16,
                             kind='ExternalInput').ap()
    d['zeros'] = nc.dram_tensor('zeros', [P, N], F16, kind='ExternalInput').ap()
    for L, I in LAYERS:
        d[L + '_wa'] = nc.dram_tensor(L + '_wa', [P, 5, P], F16,
                                      kind='ExternalInput').ap()
        d[L + '_wc0h'] = nc.dram_tensor(L + '_wc0h', [HID, 2, P], F16,
                                        kind='ExternalInput').ap()
        d[L + '_wcx'] = nc.dram_tensor(L + '_wcx', [P, 4, 2, P], F16,
                                       kind='ExternalInput').ap()
        d[L + '_wcr'] = nc.dram_tensor(L + '_wcr', [P, 4, 2, P], F16,
                                       kind='ExternalInput').ap()
        d[L + '_bru'] = nc.dram_tensor(L + '_bru', [HID, 2], F32,
                                       kind='ExternalInput').ap()
        d[L + '_bc'] = nc.dram_tensor(L + '_bc', [HID, 1], F32,
                                      kind='ExternalInput').ap()
        if I == 64:
            d[L + '_wa0x'] = nc.dram_tensor(L + '_wa0x', [HID, P], F16,
                                            kind='ExternalInput').ap()
            d[L + '_wc0x'] = nc.dram_tensor(L + '_wc0x', [HID, 2, P], F16,
                                            kind='ExternalInput').ap()
        elif L == 'enc0':
            d[L + '_wc0x'] = nc.dram_tensor(L + '_wc0x', [P, 2, P], F16,
                                            kind='ExternalInput').ap()
    d['dec0_wa0y'] = nc.dram_tensor('dec0_wa0y', [1, P], F16,
                                    kind='ExternalInput').ap()
    d['dec0_wc0y'] = nc.dram_tensor('dec0_wc0y', [1, 2, P], F16,
                                    kind='ExternalInput').ap()
    d['fcnw1'] = nc.dram_tensor('fcnw1', [HID, 1], F16, kind='ExternalInput').ap()
    d['fcnw8'] = nc.dram_tensor('fcnw8', [HID, 8], F16, kind='ExternalInput').ap()
    d['fcnb1'] = nc.dram_tensor('fcnb1', [1, 1], F32, kind='ExternalInput').ap()
    d['fcnb128'] = nc.dram_tensor('fcnb128', [P, 1], F32, kind='ExternalInput').ap()
    d_out = nc.dram_tensor('out', [HOR, CHAINS, N], F16, kind='ExternalOutput').ap()

    _temit = _time.time()
    with tile.TileContext(nc) as tc:
        with tc.tile_pool(name='const', bufs=1) as const, \
             tc.tile_pool(name='state', bufs=1) as state, \
             tc.tile_pool(name='v', bufs=14) as vp, \
             tc.tile_pool(name='ru', bufs=6) as rup, \
             tc.tile_pool(name='rh', bufs=4) as rhp, \
             tc.tile_pool(name='ct', bufs=4) as ctp, \
             tc.tile_pool(name='tg', bufs=3) as tgp, \
             tc.tile_pool(name='pd', bufs=2, space='PSUM') as pdp, \
             tc.tile_pool(name='prj', bufs=2, space='PSUM') as prj:

            # ---- load constants ----
            cst = {}
            for name, dd in d.items():
                if name in ('xT', 'xn', 'zeros'):
                    continue
                t = const.tile(list(dd.shape), dd.dtype, tag=name,
                               name='cst_' + name)
                nc.sync.dma_start(t[:], dd[:])
                cst[name] = t
            s_t = [cst[f's_t{m}'] for m in range(4)]
            ident = cst['ident']
            d_zeros = d['zeros']
            zn_zeros = d_zeros.rearrange('p (kt f) -> p kt f', kt=KT)

            # ---- persistent state ----
            zT0, zT1, zn0, zn1, ytmp = [], [], [], [], []
            for c in range(CHAINS):
                zT0.append(state.tile([P, N], F16, tag=f'zT0_{c}', name=f'zT0_{c}'))
                zT1.append(state.tile([HID, N], F16, tag=f'zT1_{c}', name=f'zT1_{c}'))
                zn0.append(state.tile([P, KT, P], F16, tag=f'zn0_{c}', name=f'zn0_{c}'))
                zn1.append(state.tile([P, KT, P], F16, tag=f'zn1_{c}', name=f'zn1_{c}'))
                ytmp.append(state.tile([1, N], F16, tag=f'ytmp_{c}', name=f'ytmp_{c}'))
                nc.sync.dma_start(zT0[c][:], d_zeros[:])
                nc.sync.dma_start(zT1[c][:], d_zeros[0:HID, :])
                nc.sync.dma_start(zn0[c][:], zn_zeros[:])
                nc.sync.dma_start(zn1[c][:], zn_zeros[:])
            zr = [state.tile([P, KT, P], F16, tag=f'zr_{pr}', name=f'zr_{pr}')
                  for pr in range(2)]

            copy_rr = [0]
            COPY_ENGINES = None  # set after nc engines known

            def copy_psum(dst, src):
                """Alternate PSUM->SBUF copies between Act and DVE."""
                copy_rr[0] += 1
                if copy_rr[0] % 2:
                    nc.scalar.activation(dst, src, AF.Copy)
                else:
                    nc.vector.tensor_copy(dst, src)

            def diffuse(zn_tile, m, tag):
                """V = (S_m z)^T feature-major [128, N] fp16 SBUF tile."""
                pd = pdp.tile([P, N], F32, tag='pd')
                stt = s_t[m]
                for ms in range(2):
                    sl = slice(ms * 512, (ms + 1) * 512)
                    for kt in range(KT):
                        nc.tensor.matmul(pd[:, sl], zn_tile[:, kt, :],
                                         stt[:, kt, sl],
                                         start=(kt == 0), stop=(kt == KT - 1))
                v = vp.tile([P, N], F16, tag=tag)
                copy_psum(v[:], pd[:])
                return v

            def cell_pair(L, I, c0, is_layer0, t_idx, phase):
                c1 = c0 + 1
                pair = c0 // 2
                wa = cst[L + '_wa']
                znL = zn0 if is_layer0 else zn1
                zTh = [zT0[c0], zT0[c1]] if is_layer0 else [zT1[c0], zT1[c1]]

                # ---- gate A: diffuse per-chain packs, project, sigmoid ----
                vts = {}
                for c in (c0, c1):
                    for m in range(4):
                        vts[(c, m)] = diffuse(znL[c], m, 'v')

                ru_r, ru_u = {}, {}
                for c in (c0, c1):
                    pa = prj.tile([P, N], F32, tag='prj')
                    for ms in range(2):
                        sl = slice(ms * 512, (ms + 1) * 512)
                        mms = []
                        if L == 'enc0':
                            mms.append((wa[:, 0, :], zT0[c][:, sl]))
                        elif L == 'dec0':
                            mms.append((wa[:, 0, :], zT0[c][:, sl]))
                            mms.append((cst['dec0_wa0y'][:], ytmp[c][:, sl]))
                        else:
                            mms.append((wa[0:HID, 0, :], zTh[c - c0][0:HID, sl]))
                            mms.append((cst[L + '_wa0x'][:], zT0[c][0:HID, sl]))
                        for m in range(4):
                            mms.append((wa[:, m + 1, :], vts[(c, m)][:, sl]))
                        for j, (lhs, rhs) in enumerate(mms):
                            nc.tensor.matmul(pa[:, sl], lhs, rhs,
                                             start=(j == 0),
                                             stop=(j == len(mms) - 1))
                    rr = rup.tile([HID, N], F16, tag='ru')
                    uu = rup.tile([HID, N], F16, tag='ru')
                    nc.scalar.activation(rr[:], pa[0:HID, :], AF.Sigmoid,
                                         bias=cst[L + '_bru'][:, 0:1])
                    nc.scalar.activation(uu[:], pa[HID:P, :], AF.Sigmoid,
                                         bias=cst[L + '_bru'][:, 1:2])
                    ru_r[c], ru_u[c] = rr, uu

                # ---- r*h, transpose pair into node-major zr pack ----
                rh = {}
                for c in (c0, c1):
                    rh_c = rhp.tile([HID, N], F16, tag='rh')
                    nc.vector.tensor_mul(rh_c[:], ru_r[c][:], zTh[c - c0][0:HID, :])
                    rh[c] = rh_c
                ptr = pdp.tile([P, KT, P], F16, tag='pd')
                for c in (c0, c1):
                    off = (c - c0) * HID
                    for kt in range(KT):
                        nc.tensor.transpose(
                            ptr[:, kt, off:off + HID],
                            rh[c][:, kt * P:(kt + 1) * P],
                            ident[0:HID, 0:HID])
                nc.vector.tensor_copy(zr[pair][:], ptr[:])

                # ---- gate B: diffuse rh pack, project (single group), tanh ----
                vps = [diffuse(zr[pair], m, 'v') for m in range(4)]
                pb = prj.tile([P, N], F32, tag='prj')
                wc0h = cst[L + '_wc0h']
                wcx = cst[L + '_wcx']
                wcr = cst[L + '_wcr']
                for ms in range(2):
                    sl = slice(ms * 512, (ms + 1) * 512)
                    mms = []
                    for c in (c0, c1):
                        hh = c - c0
                        mms.append((wc0h[:, hh, :], rh[c][:, sl]))
                        if L == 'enc0':
                            mms.append((cst[L + '_wc0x'][:, hh, :], zT0[c][:, sl]))
                        elif L == 'dec0':
                            mms.append((cst['dec0_wc0y'][:, hh, :], ytmp[c][:, sl]))
                        else:
                            mms.append((cst[L + '_wc0x'][:, hh, :],
                                        zT0[c][0:HID, sl]))
                        for m in range(4):
                            mms.append((wcx[:, m, hh, :], vts[(c, m)][:, sl]))
                            mms.append((wcr[:, m, hh, :], vps[m][:, sl]))
                    for j, (lhs, rhs) in enumerate(mms):
                        nc.tensor.matmul(pb[:, sl], lhs, rhs,
                                         start=(j == 0), stop=(j == len(mms) - 1))
                ct = {}
                for c in (c0, c1):
                    hh = c - c0
                    ct_c = ctp.tile([HID, N], F16, tag='ct')
                    nc.scalar.activation(ct_c[:], pb[hh * HID:(hh + 1) * HID, :],
                                         AF.Tanh, bias=cst[L + '_bc'][:])
                    ct[c] = ct_c

                # ---- GRU update + h' back to node-major ----
                pth = pdp.tile([P, KT, P], F16, tag='pd')
                for c in (c0, c1):
                    zh = zTh[c - c0]
                    t1 = tgp.tile([HID, N], F16, tag='tg')
                    nc.vector.tensor_sub(t1[:], zh[0:HID, :], ct[c][:])
                    nc.vector.tensor_mul(t1[:], t1[:], ru_u[c][:])
                    nc.vector.tensor_add(zh[0:HID, :], ct[c][:], t1[:])
                    off = (c - c0) * HID
                    for kt in range(KT):
                        nc.tensor.transpose(
                            pth[:, kt, off:off + HID],
                            zh[0:HID, kt * P:(kt + 1) * P],
                            ident[0:HID, 0:HID])
                for c in (c0, c1):
                    off = (c - c0) * HID
                    copy_psum(znL[c][:, :, 0:HID], pth[:, :, off:off + HID])
                    if is_layer0:
                        copy_psum(zn1[c][:, :, HID:P], pth[:, :, off:off + HID])

            # ================= encoder =================
            for t in range(T):
                for c in range(CHAINS):
                    nc.sync.dma_start(zT0[c][HID:HID + 2, :], d['xT'][c, t])
                    nc.sync.dma_start(zn0[c][:, :, HID:HID + 2], d['xn'][c, t])
                for pr in range(2):
                    cell_pair('enc0', 2, 2 * pr, True, t, 'enc')
                for pr in range(2):
                    cell_pair('enc1', 64, 2 * pr, False, t, 'enc')

            # ================= decoder =================
            for c in range(CHAINS):
                nc.sync.dma_start(zT0[c][HID:HID + 2, :], d_zeros[0:2, :])
                nc.sync.dma_start(
                    zn0[c][:, :, HID:HID + 2],
                    d_zeros[:, 0:2 * KT].rearrange('p (kt f) -> p kt f', kt=KT))
                nc.sync.dma_start(ytmp[c][:], d_zeros[0:1, :])
            for t in range(HOR):
                for pr in range(2):
                    cell_pair('dec0', 1, 2 * pr, True, t, 'dec')
                for pr in range(2):
                    cell_pair('dec1', 64, 2 * pr, False, t, 'dec')
                for c in range(CHAINS):
                    # y = h1' @ fcnW + b, feature-major [1, N]
                    py = pdp.tile([1, N], F32, tag='pd')
                    for ms in range(2):
                        sl = slice(ms * 512, (ms + 1) * 512)
                        nc.tensor.matmul(py[:, sl], cst['fcnw1'][:],
                                         zT1[c][0:HID, sl], start=True, stop=True)
                    nc.scalar.activation(ytmp[c][:], py[:], AF.Identity,
                                         bias=cst['fcnb1'][:])
                    nc.sync.dma_start(d_out[t, c:c + 1, :], ytmp[c][:])
                    if t < HOR - 1:
                        # y node-major into zn0 x-col
                        pyn = pdp.tile([P, KT, 8], F32, tag='pd')
                        for kt in range(KT):
                            nc.tensor.matmul(pyn[:, kt, :],
                                             zT1[c][0:HID, kt * P:(kt + 1) * P],
                                             cst['fcnw8'][:], start=True, stop=True)
                        nc.scalar.activation(zn0[c][:, :, HID:HID + 1],
                                             pyn[:, :, 0:1], AF.Identity,
                                             bias=cst['fcnb128'][:])

    print(f'[build] emission+schedule: {_time.time() - _t0:.1f}s', flush=True)
    _t1 = _time.time()
    nc.compile()
    print(f'[build] bacc compile: {_time.time() - _t1:.1f}s', flush=True)
    return nc


def _prep_host(inputs):
    """Host-side preprocessing -> per-core in_maps (fp16)."""
    f32 = np.float32
    f16 = np.float16
    adj = np.asarray(inputs['adj'], f32)
    source = np.asarray(inputs['source'], f32)       # [B, T, N, 2]

    def rw(a):
        return (a / np.maximum(a.sum(1, keepdims=True), np.float32(1e-8))).astype(f32)

    # rhs tiles hold S^T in [node_in, node_out]; S0 = rw(adj).T -> S0^T = rw(adj)
    s0 = rw(adj)
    s1 = rw(adj.T)
    mats = [s0, s0 @ s0, s1, s1 @ s1]

    def tile_nm(a):          # [N, N] -> [128, KT, N]
        return np.ascontiguousarray(
            a.reshape(KT, P, N).transpose(1, 0, 2)).astype(f16)

    common = {f's_t{m}': tile_nm(mats[m]) for m in range(4)}
    common['ident'] = np.eye(P, dtype=f16)

    for L, I in LAYERS:
        F = I + HID
        Wru = np.asarray(inputs[L + '_Wru'], f32)
        Wc = np.asarray(inputs[L + '_Wc'], f32)
        bA = [Wru[i * F:(i + 1) * F] for i in range(5)]
        bC = [Wc[i * F:(i + 1) * F] for i in range(5)]
        eA = [bA[0] - bA[2] - bA[4], bA[1], 2 * bA[2], bA[3], 2 * bA[4]]
        eC = [bC[0] - bC[2] - bC[4], bC[1], 2 * bC[2], bC[3], 2 * bC[4]]

        wa = np.zeros((P, 5, P), f32)
        for i in range(5):
            wa[0:HID, i] = eA[i][I:F]          # h-part rows 0:64
            if not (L == 'dec0' and i == 0):
                wa[HID:HID + I, i] = eA[i][0:I]  # x-part rows 64:64+I
        common[L + '_wa'] = wa.astype(f16)

        wc0h = np.zeros((HID, 2, P), f32)
        wcx = np.zeros((P, 4, 2, P), f32)
        wcr = np.zeros((P, 4, 2, P), f32)
        for hh in range(2):
            cs = slice(hh * HID, (hh + 1) * HID)
            wc0h[:, hh, cs] = eC[0][I:F]
            for i in range(4):
                if L != 'dec0':
                    wcx[HID:HID + I, i, hh, cs] = eC[i + 1][0:I]
                else:
                    wcx[HID:HID + 1, i, hh, cs] = eC[i + 1][0:1]
                wcr[hh * HID:(hh + 1) * HID, i, hh, cs] = eC[i + 1][I:F]
        common[L + '_wc0h'] = wc0h.astype(f16)
        common[L + '_wcx'] = wcx.astype(f16)
        common[L + '_wcr'] = wcr.astype(f16)
        bru = np.asarray(inputs[L + '_bru'], f32)
        common[L + '_bru'] = np.stack([bru[0:HID], bru[HID:P]], axis=1).copy()
        common[L + '_bc'] = np.asarray(inputs[L + '_bc'], f32).reshape(HID, 1).copy()

        if I == 64:
            common[L + '_wa0x'] = np.ascontiguousarray(eA[0][0:I]).astype(f16)
            wc0x = np.zeros((HID, 2, P), f32)
            for hh in range(2):
                wc0x[:, hh, hh * HID:(hh + 1) * HID] = eC[0][0:I]
            common[L + '_wc0x'] = wc0x.astype(f16)
        elif L == 'enc0':
            wc0x = np.zeros((P, 2, P), f32)
            for hh in range(2):
                wc0x[HID:HID + I, hh, hh * HID:(hh + 1) * HID] = eC[0][0:I]
            common[L + '_wc0x'] = wc0x.astype(f16)
        elif L == 'dec0':
            common['dec0_wa0y'] = np.ascontiguousarray(eA[0][0:1]).astype(f16)
            wc0y = np.zeros((1, 2, P), f32)
            for hh in range(2):
                wc0y[:, hh, hh * HID:(hh + 1) * HID] = eC[0][0:1]
            common['dec0_wc0y'] = wc0y.astype(f16)

    common['zeros'] = np.zeros((P, N), f16)
    fw = np.asarray(inputs['fcn_W'], f32).reshape(HID, 1)
    common['fcnw1'] = fw.astype(f16).copy()
    common['fcnw8'] = np.repeat(fw, 8, axis=1).astype(f16).copy()
    fb = np.asarray(inputs['fcn_b'], f32).reshape(-1)[0]
    common['fcnb1'] = np.full((1, 1), fb, f32)
    common['fcnb128'] = np.full((P, 1), fb, f32)

    T = _cache['T']
    in_maps = []
    for core in range(N_CORES):
        m = dict(common)
        xT = np.zeros((CHAINS, T, 2, N), f16)
        xn = np.zeros((CHAINS, T, P, KT, 2), f16)
        for c in range(CHAINS):
            b = core * CHAINS + c
            for t in range(T):
                xt = source[b, t]            # [N, 2]
                xT[c, t] = xt.T.astype(f16)
                xn[c, t] = xt.reshape(KT, P, 2).transpose(1, 0, 2).astype(f16)
        m['xT'] = xT
        m['xn'] = xn
        in_maps.append(m)
    return in_maps


def kernel(**inputs):
    from concourse import bass_utils

    T = int(os.environ.get('DCRNN_T', T_FULL))
    HOR = int(os.environ.get('DCRNN_HOR', HOR_FULL))
    key = (T, HOR)
    if _cache.get('key') != key:
        _cache['nc'] = _build(T, HOR)
        _cache['key'] = key
    _cache['T'] = T
    _cache['HOR'] = HOR

    in_maps = _prep_host(inputs)
    res = bass_utils.run_bass_kernel_spmd(
        _cache['nc'], in_maps, core_ids=list(range(N_CORES)),
        trace=bool(int(os.environ.get('DCRNN_TRACE', '0'))))
    _cache['last_res'] = res

    HORr = _cache['HOR']
    out = np.zeros((HORr, B, N), np.float32)
    for core in range(N_CORES):
        r = res.results[core]['out']         # [HOR, CHAINS, N] fp16
        for c in range(CHAINS):
            out[:, core * CHAINS + c, :] = r[:, c, :].astype(np.float32)
    return out


# revision 5
# speedup vs baseline: 2.6436x; 1.2588x over previous
"""DCRNN (nn_DCRNNModel) forward pass on 8 Trainium2 NeuronCores.

Data-parallel over batch (B=32 -> 4 chains/core). All weights plus the four
diffusion operators S0, S0^2, S1, S1^2 (S^2 precomputed on host) live in
SBUF in fp16 for the whole kernel. All PE matmuls run in fp16 (1 cycle/row
at any moving size); PSUM accumulates fp32.

Diffusion reuse: layer 1 diffuses zn1=[h1|h0'] per chain; its V tiles hold
S^k h0' in rows 64:128, which layer 0 of the NEXT step reuses as its h-part
diffusion (via row-shifted weights), so layer 0 only diffuses the tiny
x-quad pack (all 4 chains' inputs in one 8-feature pack, 4 sets/step total).
Gate B diffuses only r*h, pair-packed two chains per 128-wide pack; its
x-terms reuse the V tiles through zero-padded K=128 weights so every matmul
operand sits at partition base 0 (HW requires equal base partitions and a
stable PE tile row inside an accumulation group).
Loops are ordered so consecutive matmuls share the same stationary operand
(one weight load per kt instead of per (kt,slice)).
"""
import sys
import os
import time as _time
import numpy as np

sys.path.insert(0, '/opt/trn_rl_repo')

N = 1024
B = 32
T_FULL = 12
HOR_FULL = 12
HID = 64
N_CORES = 8
CHAINS = 4
KT = 8
P = 128

_cache = {}

L1LAYERS = (('enc1', 64), ('dec1', 64))


def _build(T, HOR):
    import concourse.bacc as bacc
    import concourse.tile as tile
    from concourse import mybir

    F32 = mybir.dt.float32
    F16 = mybir.dt.float16
    AF = mybir.ActivationFunctionType

    _t0 = _time.time()
    nc = bacc.Bacc('TRN2', target_bir_lowering=False, debug=False,
                   num_devices=N_CORES)

    # ---- DRAM declarations ----
    d = {}
    for m in range(4):
        d[f's_t{m}'] = nc.dram_tensor(f's_t{m}', [P, KT, N], F16,
                                      kind='ExternalInput').ap()
    d['ident'] = nc.dram_tensor('ident', [P, P], F16, kind='ExternalInput').ap()
    d['xT'] = nc.dram_tensor('xT', [CHAINS, T, 2, N], F16,
                             kind='ExternalInput').ap()
    d['xq'] = nc.dram_tensor('xq', [P, T, KT, 8], F16, kind='ExternalInput').ap()
    d['zeros'] = nc.dram_tensor('zeros', [P, N], F16, kind='ExternalInput').ap()
    # layer-0 weights (enc0 I=2, dec0 I=1)
    for L, I in (('enc0', 2), ('dec0', 1)):
        d[L + '_wa_id'] = nc.dram_tensor(L + '_wa_id', [P, P], F16,
                                         kind='ExternalInput').ap()
        d[L + '_wa_hs'] = nc.dram_tensor(L + '_wa_hs', [P, 4, P], F16,
                                         kind='ExternalInput').ap()
        d[L + '_waq'] = nc.dram_tensor(L + '_waq', [4 * I, 4, CHAINS, P], F16,
                                       kind='ExternalInput').ap()
        d[L + '_wcq'] = nc.dram_tensor(L + '_wcq', [4 * I, 4, CHAINS, HID], F16,
                                       kind='ExternalInput').ap()
        d[L + '_wc0h'] = nc.dram_tensor(L + '_wc0h', [HID, HID], F16,
                                        kind='ExternalInput').ap()
        d[L + '_wcr0'] = nc.dram_tensor(L + '_wcr0', [P, 4, HID], F16,
                                        kind='ExternalInput').ap()
        d[L + '_wcr1'] = nc.dram_tensor(L + '_wcr1', [P, 4, HID], F16,
                                        kind='ExternalInput').ap()
        d[L + '_bru'] = nc.dram_tensor(L + '_bru', [HID, 2], F32,
                                       kind='ExternalInput').ap()
        d[L + '_bc'] = nc.dram_tensor(L + '_bc', [HID, 1], F32,
                                      kind='ExternalInput').ap()
    d['enc0_wc0x'] = nc.dram_tensor('enc0_wc0x', [P, HID], F16,
                                    kind='ExternalInput').ap()
    d['dec0_wa0y'] = nc.dram_tensor('dec0_wa0y', [1, P], F16,
                                    kind='ExternalInput').ap()
    d['dec0_wc0y'] = nc.dram_tensor('dec0_wc0y', [1, HID], F16,
                                    kind='ExternalInput').ap()
    # layer-1 weights (I=64)
    for L, I in L1LAYERS:
        d[L + '_wa'] = nc.dram_tensor(L + '_wa', [P, 5, P], F16,
                                      kind='ExternalInput').ap()
        d[L + '_wa0x'] = nc.dram_tensor(L + '_wa0x', [HID, P], F16,
                                        kind='ExternalInput').ap()
        d[L + '_wc0h'] = nc.dram_tensor(L + '_wc0h', [HID, HID], F16,
                                        kind='ExternalInput').ap()
        d[L + '_wc0x'] = nc.dram_tensor(L + '_wc0x', [HID, HID], F16,
                                        kind='ExternalInput').ap()
        d[L + '_wcx'] = nc.dram_tensor(L + '_wcx', [P, 4, HID], F16,
                                       kind='ExternalInput').ap()
        d[L + '_wcr0'] = nc.dram_tensor(L + '_wcr0', [P, 4, HID], F16,
                                        kind='ExternalInput').ap()
        d[L + '_wcr1'] = nc.dram_tensor(L + '_wcr1', [P, 4, HID], F16,
                                        kind='ExternalInput').ap()
        d[L + '_bru'] = nc.dram_tensor(L + '_bru', [HID, 2], F32,
                                       kind='ExternalInput').ap()
        d[L + '_bc'] = nc.dram_tensor(L + '_bc', [HID, 1], F32,
                                      kind='ExternalInput').ap()
    d['fcnw1'] = nc.dram_tensor('fcnw1', [HID, 1], F16, kind='ExternalInput').ap()
    d['fcnb1'] = nc.dram_tensor('fcnb1', [1, 1], F32, kind='ExternalInput').ap()
    d['fcnb128'] = nc.dram_tensor('fcnb128', [P, 1], F32, kind='ExternalInput').ap()
    d_out = nc.dram_tensor('out', [HOR, CHAINS, N], F16, kind='ExternalOutput').ap()

    with tile.TileContext(nc) as tc:
        with tc.tile_pool(name='const', bufs=1) as const, \
             tc.tile_pool(name='state', bufs=1) as state, \
             tc.tile_pool(name='vl', bufs=18) as vlp, \
             tc.tile_pool(name='vp', bufs=5) as vpp, \
             tc.tile_pool(name='vq', bufs=4) as vqp, \
             tc.tile_pool(name='ru', bufs=5) as rup, \
             tc.tile_pool(name='rh', bufs=3) as rhp, \
             tc.tile_pool(name='ct', bufs=3) as ctp, \
             tc.tile_pool(name='tg', bufs=2) as tgp, \
             tc.tile_pool(name='pd', bufs=2, space='PSUM') as pdp, \
             tc.tile_pool(name='prj', bufs=2, space='PSUM') as prj:

            # ---- load constants ----
            cst = {}
            for name, dd in d.items():
                if name in ('xT', 'zeros'):
                    continue
                t = const.tile(list(dd.shape), dd.dtype, tag=name,
                               name='cst_' + name)
                nc.sync.dma_start(t[:], dd[:])
                cst[name] = t
            s_t = [cst[f's_t{m}'] for m in range(4)]
            ident = cst['ident']
            d_zeros = d['zeros']
            zn_zeros = d_zeros.rearrange('p (kt f) -> p kt f', kt=KT)

            # ---- persistent state ----
            zT0, zT1, zn1, ytmp = [], [], [], []
            for c in range(CHAINS):
                zT0.append(state.tile([P, N], F16, tag=f'zT0_{c}', name=f'zT0_{c}'))
                zT1.append(state.tile([HID, N], F16, tag=f'zT1_{c}', name=f'zT1_{c}'))
                zn1.append(state.tile([P, KT, P], F16, tag=f'zn1_{c}', name=f'zn1_{c}'))
                ytmp.append(state.tile([1, N], F16, tag=f'ytmp_{c}', name=f'ytmp_{c}'))
                nc.sync.dma_start(zT0[c][:], d_zeros[:])
                nc.sync.dma_start(zT1[c][:], d_zeros[0:HID, :])
                nc.sync.dma_start(zn1[c][:], zn_zeros[:])
            zr = [state.tile([P, KT, P], F16, tag=f'zr_{pr}', name=f'zr_{pr}')
                  for pr in range(2)]
            yq = state.tile([P, KT, CHAINS], F16, tag='yq', name='yq')

            vl1 = {}     # (c, m) -> V tile of layer-1 diffusion (prev step)

            copy_rr = [0]

            def copy_psum(dst, src):
                copy_rr[0] += 1
                if copy_rr[0] % 2:
                    nc.scalar.activation(dst, src, AF.Copy)
                else:
                    nc.vector.tensor_copy(dst, src)

            def diffuse(lhs_fn, m, pool, tag, nfeat=P):
                """V = (S_m z)^T feature-major [nfeat, N] fp16 SBUF tile.
                lhs_fn(kt) -> lhsT AP [128, nfeat]."""
                pd = pdp.tile([P, N], F32, tag='pd')
                stt = s_t[m]
                for kt in range(KT):
                    for ms in range(2):
                        sl = slice(ms * 512, (ms + 1) * 512)
                        nc.tensor.matmul(pd[0:nfeat, sl], lhs_fn(kt),
                                         stt[:, kt, sl],
                                         start=(kt == 0), stop=(kt == KT - 1))
                v = pool.tile([nfeat, N], F16, tag=tag)
                copy_psum(v[:], pd[0:nfeat, :])
                return v

            def proj(ptile, psl, mms):
                """Accumulate terms into ptile[psl, :]; term-outer order so
                both 512-slices of one term share the weight load."""
                nmm = len(mms)
                for j, (lhs, rhs) in enumerate(mms):
                    for ms in range(2):
                        sl = slice(ms * 512, (ms + 1) * 512)
                        nc.tensor.matmul(ptile[psl, sl], lhs, rhs[:, sl],
                                         start=(j == 0), stop=(j == nmm - 1))

            def sigmoid_ru(L, pa):
                rr = rup.tile([HID, N], F16, tag='ru')
                uu = rup.tile([HID, N], F16, tag='ru')
                nc.scalar.activation(rr[:], pa[0:HID, :], AF.Sigmoid,
                                     bias=cst[L + '_bru'][:, 0:1])
                nc.scalar.activation(uu[:], pa[HID:P, :], AF.Sigmoid,
                                     bias=cst[L + '_bru'][:, 1:2])
                return rr, uu

            def gru_and_transpose(zh, ct_c, uu, dst_znc, dst_off):
                """h' = c + u*(h-c); transpose h' into dst_znc[:, :, off]."""
                t1 = tgp.tile([HID, N], F16, tag='tg')
                nc.vector.tensor_sub(t1[:], zh[0:HID, :], ct_c[:])
                nc.vector.tensor_mul(t1[:], t1[:], uu[:])
                nc.vector.tensor_add(zh[0:HID, :], ct_c[:], t1[:])
                pth = pdp.tile([P, KT, HID], F16, tag='pd')
                for kt in range(KT):
                    nc.tensor.transpose(pth[:, kt, :],
                                        zh[0:HID, kt * P:(kt + 1) * P],
                                        ident[0:HID, 0:HID])
                copy_psum(dst_znc[:, :, dst_off:dst_off + HID], pth[:])

            def gateB(L, pair, c0, rh, vq, vx):
                """Gate B: transpose+diffuse rh pair-pack, per-chain proj+tanh.
                vq: 4 quad V tiles (layer0) or None; vx: dict (c,m)->V tile
                whose rows 64:128 hold the x-part diffusion (layer1)."""
                c1 = c0 + 1
                ptr = pdp.tile([P, KT, P], F16, tag='pd')
                for c in (c0, c1):
                    off = (c - c0) * HID
                    for kt in range(KT):
                        nc.tensor.transpose(ptr[:, kt, off:off + HID],
                                            rh[c][:, kt * P:(kt + 1) * P],
                                            ident[0:HID, 0:HID])
                nc.vector.tensor_copy(zr[pair][:], ptr[:])
                vps = [diffuse(lambda kt, pr=pair: zr[pr][:, kt, :], m, vpp, 'vp')
                       for m in range(4)]
                ct = {}
                for c in (c0, c1):
                    hh = c - c0
                    wcr = cst[L + ('_wcr0', '_wcr1')[hh]]
                    pb = prj.tile([P, N], F32, tag='prj')
                    mms = [(cst[L + '_wc0h'][:], rh[c])]
                    if L == 'enc0':
                        mms.append((cst['enc0_wc0x'][:], zT0[c]))
                    elif L == 'dec0':
                        mms.append((cst['dec0_wc0y'][:], ytmp[c]))
                    else:
                        mms.append((cst[L + '_wc0x'][:], zT0[c][0:HID, :]))
                    for m in range(4):
                        if vq is not None:
                            mms.append((cst[L + '_wcq'][:, m, c, :], vq[m]))
                        else:
                            mms.append((cst[L + '_wcx'][:, m, :], vx[(c, m)]))
                        mms.append((wcr[:, m, :], vps[m]))
                    proj(pb, slice(0, HID), mms)
                    ct_c = ctp.tile([HID, N], F16, tag='ct')
                    nc.scalar.activation(ct_c[:], pb[0:HID, :], AF.Tanh,
                                         bias=cst[L + '_bc'][:])
                    ct[c] = ct_c
                return ct

            def cell_pair_l0(L, c0, vq, skip_h):
                """Layer-0 cell for chains (c0, c0+1): no own h-diffusion."""
                c1 = c0 + 1
                ru_r, ru_u = {}, {}
                for c in (c0, c1):
                    pa = prj.tile([P, N], F32, tag='prj')
                    mms = [(cst[L + '_wa_id'][:], zT0[c])]
                    if L == 'dec0':
                        mms.append((cst['dec0_wa0y'][:], ytmp[c]))
                    if not skip_h:
                        for m in range(4):
                            mms.append((cst[L + '_wa_hs'][:, m, :], vl1[(c, m)]))
                    for m in range(4):
                        mms.append((cst[L + '_waq'][:, m, c, :], vq[m]))
                    proj(pa, slice(0, P), mms)
                    ru_r[c], ru_u[c] = sigmoid_ru(L, pa)
                rh = {}
                for c in (c0, c1):
                    rh_c = rhp.tile([HID, N], F16, tag='rh')
                    nc.vector.tensor_mul(rh_c[:], ru_r[c][:], zT0[c][0:HID, :])
                    rh[c] = rh_c
                ct = gateB(L, c0 // 2, c0, rh, vq, None)
                for c in (c0, c1):
                    gru_and_transpose(zT0[c], ct[c], ru_u[c], zn1[c], HID)

            def cell_pair_l1(L, c0):
                """Layer-1 cell for chains (c0, c0+1): diffuses zn1=[h1|h0']."""
                c1 = c0 + 1
                wa = cst[L + '_wa']
                for c in (c0, c1):
                    for m in range(4):
                        vl1[(c, m)] = diffuse(
                            lambda kt, c=c: zn1[c][:, kt, :], m, vlp, 'vl')
                ru_r, ru_u = {}, {}
                for c in (c0, c1):
                    pa = prj.tile([P, N], F32, tag='prj')
                    mms = [(wa[0:HID, 0, :], zT1[c][0:HID, :]),
                           (cst[L + '_wa0x'][:], zT0[c][0:HID, :])]
                    for m in range(4):
                        mms.append((wa[:, m + 1, :], vl1[(c, m)]))
                    proj(pa, slice(0, P), mms)
                    ru_r[c], ru_u[c] = sigmoid_ru(L, pa)
                rh = {}
                for c in (c0, c1):
                    rh_c = rhp.tile([HID, N], F16, tag='rh')
                    nc.vector.tensor_mul(rh_c[:], ru_r[c][:], zT1[c][0:HID, :])
                    rh[c] = rh_c
                ct = gateB(L, c0 // 2, c0, rh, None, vl1)
                for c in (c0, c1):
                    gru_and_transpose(zT1[c], ct[c], ru_u[c], zn1[c], 0)

            # ================= encoder =================
            xq_t = cst['xq']
            for t in range(T):
                for c in range(CHAINS):
                    nc.sync.dma_start(zT0[c][HID:HID + 2, :], d['xT'][c, t])
                vq = [diffuse(lambda kt, t=t: xq_t[:, t, kt, :], m,
                              vqp, 'vq', nfeat=8) for m in range(4)]
                for pr in range(2):
                    cell_pair_l0('enc0', 2 * pr, vq, skip_h=(t == 0))
                for pr in range(2):
                    cell_pair_l1('enc1', 2 * pr)

            # ================= decoder =================
            for c in range(CHAINS):
                nc.sync.dma_start(zT0[c][HID:HID + 2, :], d_zeros[0:2, :])
                nc.sync.dma_start(ytmp[c][:], d_zeros[0:1, :])
            nc.sync.dma_start(
                yq[:], d_zeros[:, 0:KT * CHAINS].rearrange(
                    'p (kt f) -> p kt f', kt=KT))
            for t in range(HOR):
                vq = [diffuse(lambda kt: yq[:, kt, :], m, vqp, 'vq',
                              nfeat=CHAINS) for m in range(4)]
                for pr in range(2):
                    cell_pair_l0('dec0', 2 * pr, vq, skip_h=False)
                for pr in range(2):
                    cell_pair_l1('dec1', 2 * pr)
                pq = pdp.tile([P, KT, CHAINS], F32, tag='pd')
                for c in range(CHAINS):
                    py = pdp.tile([1, N], F32, tag='pd')
                    for ms in range(2):
                        sl = slice(ms * 512, (ms + 1) * 512)
                        nc.tensor.matmul(py[:, sl], cst['fcnw1'][:],
                                         zT1[c][0:HID, sl], start=True, stop=True)
                    nc.scalar.activation(ytmp[c][:], py[:], AF.Identity,
                                         bias=cst['fcnb1'][:])
                    nc.sync.dma_start(d_out[t, c:c + 1, :], ytmp[c][:])
                    if t < HOR - 1:
                        for kt in range(KT):
                            nc.tensor.matmul(pq[:, kt, c:c + 1],
                                             zT1[c][0:HID, kt * P:(kt + 1) * P],
                                             cst['fcnw1'][:], start=True, stop=True)
                if t < HOR - 1:
                    nc.scalar.activation(yq[:], pq[:], AF.Identity,
                                         bias=cst['fcnb128'][:])

    print(f'[build] emission+schedule: {_time.time() - _t0:.1f}s', flush=True)
    _t1 = _time.time()
    nc.compile()
    print(f'[build] bacc compile: {_time.time() - _t1:.1f}s', flush=True)
    return nc


def _prep_host(inputs):
    """Host-side preprocessing -> per-core in_maps (fp16)."""
    f32 = np.float32
    f16 = np.float16
    adj = np.asarray(inputs['adj'], f32)
    source = np.asarray(inputs['source'], f32)       # [B, T, N, 2]

    def rw(a):
        return (a / np.maximum(a.sum(1, keepdims=True), np.float32(1e-8))).astype(f32)

    s0 = rw(adj)
    s1 = rw(adj.T)
    mats = [s0, s0 @ s0, s1, s1 @ s1]

    def tile_nm(a):
        return np.ascontiguousarray(
            a.reshape(KT, P, N).transpose(1, 0, 2)).astype(f16)

    common = {f's_t{m}': tile_nm(mats[m]) for m in range(4)}
    common['ident'] = np.eye(P, dtype=f16)

    def basis(Wname, I):
        F = I + HID
        W = np.asarray(inputs[Wname], f32)
        b = [W[i * F:(i + 1) * F] for i in range(5)]
        return [b[0] - b[2] - b[4], b[1], 2 * b[2], b[3], 2 * b[4]]

    for L, I in (('enc0', 2), ('dec0', 1)):
        F = I + HID
        eA = basis(L + '_Wru', I)
        eC = basis(L + '_Wc', I)
        wa_id = np.zeros((P, P), f32)
        wa_id[0:HID] = eA[0][I:F]
        if L == 'enc0':
            wa_id[HID:HID + I] = eA[0][0:I]
        common[L + '_wa_id'] = wa_id.astype(f16)
        wa_hs = np.zeros((P, 4, P), f32)
        for m in range(4):
            wa_hs[HID:P, m] = eA[m + 1][I:F]
        common[L + '_wa_hs'] = wa_hs.astype(f16)
        waq = np.zeros((4 * I, 4, CHAINS, P), f32)
        wcq = np.zeros((4 * I, 4, CHAINS, HID), f32)
        for m in range(4):
            for c in range(CHAINS):
                waq[c * I:(c + 1) * I, m, c] = eA[m + 1][0:I]
                wcq[c * I:(c + 1) * I, m, c] = eC[m + 1][0:I]
        common[L + '_waq'] = waq.astype(f16)
        common[L + '_wcq'] = wcq.astype(f16)
        common[L + '_wc0h'] = np.ascontiguousarray(eC[0][I:F]).astype(f16)
        for hh in range(2):
            wcr = np.zeros((P, 4, HID), f32)
            for m in range(4):
                wcr[hh * HID:(hh + 1) * HID, m] = eC[m + 1][I:F]
            common[L + f'_wcr{hh}'] = wcr.astype(f16)
        bru = np.asarray(inputs[L + '_bru'], f32)
        common[L + '_bru'] = np.stack([bru[0:HID], bru[HID:P]], axis=1).copy()
        common[L + '_bc'] = np.asarray(inputs[L + '_bc'], f32).reshape(HID, 1).copy()
        if L == 'enc0':
            wc0x = np.zeros((P, HID), f32)
            wc0x[HID:HID + I] = eC[0][0:I]
            common['enc0_wc0x'] = wc0x.astype(f16)
        else:
            common['dec0_wa0y'] = np.ascontiguousarray(eA[0][0:1]).astype(f16)
            common['dec0_wc0y'] = np.ascontiguousarray(eC[0][0:1]).astype(f16)

    for L, I in L1LAYERS:
        F = I + HID
        eA = basis(L + '_Wru', I)
        eC = basis(L + '_Wc', I)
        wa = np.zeros((P, 5, P), f32)
        for i in range(5):
            wa[0:HID, i] = eA[i][I:F]
            if i > 0:
                wa[HID:P, i] = eA[i][0:I]
        common[L + '_wa'] = wa.astype(f16)
        common[L + '_wa0x'] = np.ascontiguousarray(eA[0][0:I]).astype(f16)
        common[L + '_wc0h'] = np.ascontiguousarray(eC[0][I:F]).astype(f16)
        common[L + '_wc0x'] = np.ascontiguousarray(eC[0][0:I]).astype(f16)
        wcx = np.zeros((P, 4, HID), f32)
        for m in range(4):
            wcx[HID:P, m] = eC[m + 1][0:I]
        common[L + '_wcx'] = wcx.astype(f16)
        for hh in range(2):
            wcr = np.zeros((P, 4, HID), f32)
            for m in range(4):
                wcr[hh * HID:(hh + 1) * HID, m] = eC[m + 1][I:F]
            common[L + f'_wcr{hh}'] = wcr.astype(f16)
        bru = np.asarray(inputs[L + '_bru'], f32)
        common[L + '_bru'] = np.stack([bru[0:HID], bru[HID:P]], axis=1).copy()
        common[L + '_bc'] = np.asarray(inputs[L + '_bc'], f32).reshape(HID, 1).copy()

    common['zeros'] = np.zeros((P, N), f16)
    fw = np.asarray(inputs['fcn_W'], f32).reshape(HID, 1)
    common['fcnw1'] = fw.astype(f16).copy()
    fb = np.asarray(inputs['fcn_b'], f32).reshape(-1)[0]
    common['fcnb1'] = np.full((1, 1), fb, f32)
    common['fcnb128'] = np.full((P, 1), fb, f32)

    T = _cache['T']
    in_maps = []
    for core in range(N_CORES):
        m = dict(common)
        xT = np.zeros((CHAINS, T, 2, N), f16)
        xq = np.zeros((P, T, KT, 8), f16)
        for c in range(CHAINS):
            b = core * CHAINS + c
            for t in range(T):
                xt = source[b, t]            # [N, 2]
                xT[c, t] = xt.T.astype(f16)
                xq[:, t, :, 2 * c:2 * c + 2] = \
                    xt.reshape(KT, P, 2).transpose(1, 0, 2).astype(f16)
        m['xT'] = xT
        m['xq'] = xq
        in_maps.append(m)
    return in_maps


def kernel(**inputs):
    from concourse import bass_utils

    T = int(os.environ.get('DCRNN_T', T_FULL))
    HOR = int(os.environ.get('DCRNN_HOR', HOR_FULL))
    key = (T, HOR)
    if _cache.get('key') != key:
        _cache['nc'] = _build(T, HOR)
        _cache['key'] = key
    _cache['T'] = T
    _cache['HOR'] = HOR

    in_maps = _prep_host(inputs)
    res = bass_utils.run_bass_kernel_spmd(
        _cache['nc'], in_maps, core_ids=list(range(N_CORES)),
        trace=bool(int(os.environ.get('DCRNN_TRACE', '0'))))
    _cache['last_res'] = res

    HORr = _cache['HOR']
    out = np.zeros((HORr, B, N), np.float32)
    for core in range(N_CORES):
        r = res.results[core]['out']         # [HOR, CHAINS, N] fp16
        for c in range(CHAINS):
            out[:, core * CHAINS + c, :] = r[:, c, :].astype(np.float32)
    return out


# revision 10
# speedup vs baseline: 2.7277x; 1.0318x over previous
"""DCRNN (nn_DCRNNModel) forward pass on 8 Trainium2 NeuronCores.

Data-parallel over batch (B=32 -> 4 chains/core). All weights plus the four
diffusion operators S0, S0^2, S1, S1^2 (S^2 precomputed on host) live in
SBUF in fp16 for the whole kernel. All PE matmuls run in fp16 (1 cycle/row
at any moving size); PSUM accumulates fp32.

Diffusion reuse: layer 1 diffuses zn1=[h1|h0'] per chain; its V tiles hold
S^k h0' in rows 64:128, which layer 0 of the NEXT step reuses as its h-part
diffusion (via row-shifted weights), so layer 0 only diffuses the tiny
x-quad pack (all 4 chains' inputs in one 8-feature pack, 4 sets/step total).
Gate B diffuses only r*h, pair-packed two chains per 128-wide pack; its
x-terms reuse the V tiles through zero-padded K=128 weights so every matmul
operand sits at partition base 0 (HW requires equal base partitions and a
stable PE tile row inside an accumulation group).
Loops are ordered so consecutive matmuls share the same stationary operand
(one weight load per kt instead of per (kt,slice)).
"""
import sys
import os
import time as _time
import numpy as np

sys.path.insert(0, '/opt/trn_rl_repo')

N = 1024
B = 32
T_FULL = 12
HOR_FULL = 12
HID = 64
N_CORES = 8
CHAINS = 4
KT = 8
P = 128

_cache = {}

L1LAYERS = (('enc1', 64), ('dec1', 64))


def _build(T, HOR):
    import concourse.bacc as bacc
    import concourse.tile as tile
    from concourse import mybir

    F32 = mybir.dt.float32
    F16 = mybir.dt.float16
    AF = mybir.ActivationFunctionType

    _t0 = _time.time()
    nc = bacc.Bacc('TRN2', target_bir_lowering=False, debug=False,
                   num_devices=N_CORES)

    # ---- DRAM declarations ----
    d = {}
    for m in range(4):
        d[f's_t{m}'] = nc.dram_tensor(f's_t{m}', [P, KT, N], F16,
                                      kind='ExternalInput').ap()
    d['ident'] = nc.dram_tensor('ident', [HID, HID], F16, kind='ExternalInput').ap()
    d['xT'] = nc.dram_tensor('xT', [CHAINS, T, 2, N], F16,
                             kind='ExternalInput').ap()
    d['xq'] = nc.dram_tensor('xq', [P, T, KT, 8], F16, kind='ExternalInput').ap()
    d['zeros'] = nc.dram_tensor('zeros', [P, N], F16, kind='ExternalInput').ap()
    # layer-0 weights (enc0 I=2, dec0 I=1)
    for L, I in (('enc0', 2), ('dec0', 1)):
        d[L + '_wa_id'] = nc.dram_tensor(L + '_wa_id', [P, P], F16,
                                         kind='ExternalInput').ap()
        d[L + '_wa_hs'] = nc.dram_tensor(L + '_wa_hs', [P, 4, P], F16,
                                         kind='ExternalInput').ap()
        d[L + '_waq'] = nc.dram_tensor(L + '_waq', [4 * I, 4, CHAINS, P], F16,
                                       kind='ExternalInput').ap()
        d[L + '_wcq'] = nc.dram_tensor(L + '_wcq', [4 * I, 4, CHAINS, HID], F16,
                                       kind='ExternalInput').ap()
        d[L + '_wc0h'] = nc.dram_tensor(L + '_wc0h', [HID, HID], F16,
                                        kind='ExternalInput').ap()
        d[L + '_wcr0'] = nc.dram_tensor(L + '_wcr0', [P, 4, HID], F16,
                                        kind='ExternalInput').ap()
        d[L + '_wcr1'] = nc.dram_tensor(L + '_wcr1', [P, 4, HID], F16,
                                        kind='ExternalInput').ap()
        d[L + '_bru'] = nc.dram_tensor(L + '_bru', [HID, 2], F32,
                                       kind='ExternalInput').ap()
        d[L + '_bc'] = nc.dram_tensor(L + '_bc', [HID, 1], F32,
                                      kind='ExternalInput').ap()
    d['enc0_wc0x'] = nc.dram_tensor('enc0_wc0x', [P, HID], F16,
                                    kind='ExternalInput').ap()
    d['dec0_wa0y'] = nc.dram_tensor('dec0_wa0y', [1, P], F16,
                                    kind='ExternalInput').ap()
    d['dec0_wc0y'] = nc.dram_tensor('dec0_wc0y', [1, HID], F16,
                                    kind='ExternalInput').ap()
    # layer-1 weights (I=64)
    for L, I in L1LAYERS:
        d[L + '_wa'] = nc.dram_tensor(L + '_wa', [P, 5, P], F16,
                                      kind='ExternalInput').ap()
        d[L + '_wa0x'] = nc.dram_tensor(L + '_wa0x', [HID, P], F16,
                                        kind='ExternalInput').ap()
        d[L + '_wc0h'] = nc.dram_tensor(L + '_wc0h', [HID, HID], F16,
                                        kind='ExternalInput').ap()
        d[L + '_wc0x'] = nc.dram_tensor(L + '_wc0x', [HID, HID], F16,
                                        kind='ExternalInput').ap()
        d[L + '_wcx'] = nc.dram_tensor(L + '_wcx', [P, 4, HID], F16,
                                       kind='ExternalInput').ap()
        d[L + '_wcr0'] = nc.dram_tensor(L + '_wcr0', [P, 4, HID], F16,
                                        kind='ExternalInput').ap()
        d[L + '_wcr1'] = nc.dram_tensor(L + '_wcr1', [P, 4, HID], F16,
                                        kind='ExternalInput').ap()
        d[L + '_bru'] = nc.dram_tensor(L + '_bru', [HID, 2], F32,
                                       kind='ExternalInput').ap()
        d[L + '_bc'] = nc.dram_tensor(L + '_bc', [HID, 1], F32,
                                      kind='ExternalInput').ap()
    d['fcnw1'] = nc.dram_tensor('fcnw1', [HID, 1], F16, kind='ExternalInput').ap()
    d['fcnb1'] = nc.dram_tensor('fcnb1', [1, 1], F32, kind='ExternalInput').ap()
    d['fcnb128'] = nc.dram_tensor('fcnb128', [P, 1], F32, kind='ExternalInput').ap()
    d_out = nc.dram_tensor('out', [HOR, CHAINS, N], F16, kind='ExternalOutput').ap()

    with tile.TileContext(nc) as tc:
        with tc.tile_pool(name='const', bufs=1) as const, \
             tc.tile_pool(name='state', bufs=1) as state, \
             tc.tile_pool(name='vl', bufs=18) as vlp, \
             tc.tile_pool(name='vp', bufs=4) as vpp, \
             tc.tile_pool(name='vq', bufs=7) as vqp, \
             tc.tile_pool(name='ru', bufs=4) as rup, \
             tc.tile_pool(name='rh', bufs=3) as rhp, \
             tc.tile_pool(name='ct', bufs=3) as ctp, \
             tc.tile_pool(name='tg', bufs=2) as tgp, \
             tc.tile_pool(name='pd', bufs=2, space='PSUM') as pdp, \
             tc.tile_pool(name='prj', bufs=2, space='PSUM') as prj:

            # ---- load constants ----
            cst = {}
            for name, dd in d.items():
                if name in ('xT', 'zeros'):
                    continue
                t = const.tile(list(dd.shape), dd.dtype, tag=name,
                               name='cst_' + name)
                nc.sync.dma_start(t[:], dd[:])
                cst[name] = t
            s_t = [cst[f's_t{m}'] for m in range(4)]
            ident = cst['ident']
            d_zeros = d['zeros']
            zn_zeros = d_zeros.rearrange('p (kt f) -> p kt f', kt=KT)

            # ---- persistent state ----
            zT0, zT1, zn1, ytmp = [], [], [], []
            for c in range(CHAINS):
                zT0.append(state.tile([P, N], F16, tag=f'zT0_{c}', name=f'zT0_{c}'))
                zT1.append(state.tile([HID, N], F16, tag=f'zT1_{c}', name=f'zT1_{c}'))
                zn1.append(state.tile([P, KT, P], F16, tag=f'zn1_{c}', name=f'zn1_{c}'))
                ytmp.append(state.tile([1, N], F16, tag=f'ytmp_{c}', name=f'ytmp_{c}'))
                nc.sync.dma_start(zT0[c][:], d_zeros[:])
                nc.sync.dma_start(zT1[c][:], d_zeros[0:HID, :])
                nc.sync.dma_start(zn1[c][:], zn_zeros[:])
            zr = [state.tile([P, KT, P], F16, tag=f'zr_{pr}', name=f'zr_{pr}')
                  for pr in range(2)]
            yq = state.tile([P, KT, CHAINS], F16, tag='yq', name='yq')

            vl1 = {}     # (c, m) -> V tile of layer-1 diffusion (prev step)

            copy_rr = [0]

            def copy_psum(dst, src):
                copy_rr[0] += 1
                if copy_rr[0] % 2:
                    nc.scalar.activation(dst, src, AF.Copy)
                else:
                    nc.vector.tensor_copy(dst, src)

            def diffuse(lhs_fn, m, pool, tag, nfeat=P):
                """V = (S_m z)^T feature-major [nfeat, N] fp16 SBUF tile.
                lhs_fn(kt) -> lhsT AP [128, nfeat]."""
                pd = pdp.tile([P, N], F32, tag='pd')
                stt = s_t[m]
                for kt in range(KT):
                    for ms in range(2):
                        sl = slice(ms * 512, (ms + 1) * 512)
                        nc.tensor.matmul(pd[0:nfeat, sl], lhs_fn(kt),
                                         stt[:, kt, sl],
                                         start=(kt == 0), stop=(kt == KT - 1))
                v = pool.tile([nfeat, N], F16, tag=tag)
                copy_psum(v[:], pd[0:nfeat, :])
                return v

            def proj(ptile, psl, mms):
                """Accumulate terms into ptile[psl, :]; term-outer order so
                both 512-slices of one term share the weight load."""
                nmm = len(mms)
                for j, (lhs, rhs) in enumerate(mms):
                    for ms in range(2):
                        sl = slice(ms * 512, (ms + 1) * 512)
                        nc.tensor.matmul(ptile[psl, sl], lhs, rhs[:, sl],
                                         start=(j == 0), stop=(j == nmm - 1))

            def sigmoid_ru(L, pa):
                rr = rup.tile([HID, N], F16, tag='ru')
                uu = rup.tile([HID, N], F16, tag='ru')
                nc.scalar.activation(rr[:], pa[0:HID, :], AF.Sigmoid,
                                     bias=cst[L + '_bru'][:, 0:1])
                nc.scalar.activation(uu[:], pa[HID:P, :], AF.Sigmoid,
                                     bias=cst[L + '_bru'][:, 1:2])
                return rr, uu

            def gru_and_transpose(zh, ct_c, uu, dst_znc, dst_off):
                """h' = c + u*(h-c); transpose h' into dst_znc[:, :, off]."""
                t1 = tgp.tile([HID, N], F16, tag='tg')
                nc.vector.tensor_sub(t1[:], zh[0:HID, :], ct_c[:])
                nc.vector.tensor_mul(t1[:], t1[:], uu[:])
                nc.vector.tensor_add(zh[0:HID, :], ct_c[:], t1[:])
                pth = pdp.tile([P, KT, HID], F16, tag='pd')
                for kt in range(KT):
                    nc.tensor.transpose(pth[:, kt, :],
                                        zh[0:HID, kt * P:(kt + 1) * P],
                                        ident[:])
                copy_psum(dst_znc[:, :, dst_off:dst_off + HID], pth[:])

            def gateB(L, pair, c0, rh, vq, vx):
                """Gate B: transpose+diffuse rh pair-pack, per-chain proj+tanh.
                vq: 4 quad V tiles (layer0) or None; vx: dict (c,m)->V tile
                whose rows 64:128 hold the x-part diffusion (layer1)."""
                c1 = c0 + 1
                ptr = pdp.tile([P, KT, P], F16, tag='pd')
                for c in (c0, c1):
                    off = (c - c0) * HID
                    for kt in range(KT):
                        nc.tensor.transpose(ptr[:, kt, off:off + HID],
                                            rh[c][:, kt * P:(kt + 1) * P],
                                            ident[:])
                nc.vector.tensor_copy(zr[pair][:], ptr[:])
                vps = [diffuse(lambda kt, pr=pair: zr[pr][:, kt, :], m, vpp, 'vp')
                       for m in range(4)]
                ct = {}
                for c in (c0, c1):
                    hh = c - c0
                    wcr = cst[L + ('_wcr0', '_wcr1')[hh]]
                    pb = prj.tile([P, N], F32, tag='prj')
                    mms = [(cst[L + '_wc0h'][:], rh[c])]
                    if L == 'enc0':
                        mms.append((cst['enc0_wc0x'][:], zT0[c]))
                    elif L == 'dec0':
                        mms.append((cst['dec0_wc0y'][:], ytmp[c]))
                    else:
                        mms.append((cst[L + '_wc0x'][:], zT0[c][0:HID, :]))
                    for m in range(4):
                        if vq is not None:
                            mms.append((cst[L + '_wcq'][:, m, c, :], vq[m]))
                        else:
                            mms.append((cst[L + '_wcx'][:, m, :], vx[(c, m)]))
                        mms.append((wcr[:, m, :], vps[m]))
                    proj(pb, slice(0, HID), mms)
                    ct_c = ctp.tile([HID, N], F16, tag='ct')
                    nc.scalar.activation(ct_c[:], pb[0:HID, :], AF.Tanh,
                                         bias=cst[L + '_bc'][:])
                    ct[c] = ct_c
                return ct

            def cell_pair_l0(L, c0, vq, skip_h):
                """Layer-0 cell for chains (c0, c0+1): no own h-diffusion."""
                c1 = c0 + 1
                ru_r, ru_u = {}, {}
                for c in (c0, c1):
                    pa = prj.tile([P, N], F32, tag='prj')
                    mms = [(cst[L + '_wa_id'][:], zT0[c])]
                    if L == 'dec0':
                        mms.append((cst['dec0_wa0y'][:], ytmp[c]))
                    if not skip_h:
                        for m in range(4):
                            mms.append((cst[L + '_wa_hs'][:, m, :], vl1[(c, m)]))
                    for m in range(4):
                        mms.append((cst[L + '_waq'][:, m, c, :], vq[m]))
                    proj(pa, slice(0, P), mms)
                    ru_r[c], ru_u[c] = sigmoid_ru(L, pa)
                rh = {}
                for c in (c0, c1):
                    rh_c = rhp.tile([HID, N], F16, tag='rh')
                    nc.vector.tensor_mul(rh_c[:], ru_r[c][:], zT0[c][0:HID, :])
                    rh[c] = rh_c
                ct = gateB(L, c0 // 2, c0, rh, vq, None)
                for c in (c0, c1):
                    gru_and_transpose(zT0[c], ct[c], ru_u[c], zn1[c], HID)

            def cell_pair_l1(L, c0):
                """Layer-1 cell for chains (c0, c0+1): diffuses zn1=[h1|h0']."""
                c1 = c0 + 1
                wa = cst[L + '_wa']
                for c in (c0, c1):
                    for m in range(4):
                        vl1[(c, m)] = diffuse(
                            lambda kt, c=c: zn1[c][:, kt, :], m, vlp, 'vl')
                ru_r, ru_u = {}, {}
                for c in (c0, c1):
                    pa = prj.tile([P, N], F32, tag='prj')
                    mms = [(wa[0:HID, 0, :], zT1[c][0:HID, :]),
                           (cst[L + '_wa0x'][:], zT0[c][0:HID, :])]
                    for m in range(4):
                        mms.append((wa[:, m + 1, :], vl1[(c, m)]))
                    proj(pa, slice(0, P), mms)
                    ru_r[c], ru_u[c] = sigmoid_ru(L, pa)
                rh = {}
                for c in (c0, c1):
                    rh_c = rhp.tile([HID, N], F16, tag='rh')
                    nc.vector.tensor_mul(rh_c[:], ru_r[c][:], zT1[c][0:HID, :])
                    rh[c] = rh_c
                ct = gateB(L, c0 // 2, c0, rh, None, vl1)
                for c in (c0, c1):
                    gru_and_transpose(zT1[c], ct[c], ru_u[c], zn1[c], 0)

            # ================= encoder =================
            xq_t = cst['xq']

            def enc_vq(t):
                return [diffuse(lambda kt, t=t: xq_t[:, t, kt, :], m,
                                vqp, 'vq', nfeat=8) for m in range(4)]

            vq = enc_vq(0)
            for t in range(T):
                for c in range(CHAINS):
                    nc.sync.dma_start(zT0[c][HID:HID + 2, :], d['xT'][c, t])
                for pr in range(2):
                    cell_pair_l0('enc0', 2 * pr, vq, skip_h=(t == 0))
                cell_pair_l1('enc1', 0)
                if t + 1 < T:
                    vq = enc_vq(t + 1)
                cell_pair_l1('enc1', 2)

            # ================= decoder =================
            for c in range(CHAINS):
                nc.sync.dma_start(zT0[c][HID:HID + 2, :], d_zeros[0:2, :])
                nc.sync.dma_start(ytmp[c][:], d_zeros[0:1, :])
            nc.sync.dma_start(
                yq[:], d_zeros[:, 0:KT * CHAINS].rearrange(
                    'p (kt f) -> p kt f', kt=KT))
            for t in range(HOR):
                vq = [diffuse(lambda kt: yq[:, kt, :], m, vqp, 'vq',
                              nfeat=CHAINS) for m in range(4)]
                for pr in range(2):
                    cell_pair_l0('dec0', 2 * pr, vq, skip_h=False)
                for pr in range(2):
                    cell_pair_l1('dec1', 2 * pr)
                    pq = pdp.tile([P, KT, 2], F32, tag='pd')
                    for c in (2 * pr, 2 * pr + 1):
                        py = pdp.tile([1, N], F32, tag='pd')
                        for ms in range(2):
                            sl = slice(ms * 512, (ms + 1) * 512)
                            nc.tensor.matmul(py[:, sl], cst['fcnw1'][:],
                                             zT1[c][0:HID, sl],
                                             start=True, stop=True)
                        nc.scalar.activation(ytmp[c][:], py[:], AF.Identity,
                                             bias=cst['fcnb1'][:])
                        nc.sync.dma_start(d_out[t, c:c + 1, :], ytmp[c][:])
                        if t < HOR - 1:
                            for kt in range(KT):
                                nc.tensor.matmul(
                                    pq[:, kt, c - 2 * pr:c - 2 * pr + 1],
                                    zT1[c][0:HID, kt * P:(kt + 1) * P],
                                    cst['fcnw1'][:], start=True, stop=True)
                    if t < HOR - 1:
                        nc.scalar.activation(yq[:, :, 2 * pr:2 * pr + 2],
                                             pq[:], AF.Identity,
                                             bias=cst['fcnb128'][:])

    print(f'[build] emission+schedule: {_time.time() - _t0:.1f}s', flush=True)
    _t1 = _time.time()
    nc.compile()
    print(f'[build] bacc compile: {_time.time() - _t1:.1f}s', flush=True)
    return nc


def _prep_host(inputs):
    """Host-side preprocessing -> per-core in_maps (fp16)."""
    f32 = np.float32
    f16 = np.float16
    adj = np.asarray(inputs['adj'], f32)
    source = np.asarray(inputs['source'], f32)       # [B, T, N, 2]

    def rw(a):
        return (a / np.maximum(a.sum(1, keepdims=True), np.float32(1e-8))).astype(f32)

    s0 = rw(adj)
    s1 = rw(adj.T)
    mats = [s0, s0 @ s0, s1, s1 @ s1]

    def tile_nm(a):
        return np.ascontiguousarray(
            a.reshape(KT, P, N).transpose(1, 0, 2)).astype(f16)

    common = {f's_t{m}': tile_nm(mats[m]) for m in range(4)}
    common['ident'] = np.eye(HID, dtype=f16)

    def basis(Wname, I):
        F = I + HID
        W = np.asarray(inputs[Wname], f32)
        b = [W[i * F:(i + 1) * F] for i in range(5)]
        return [b[0] - b[2] - b[4], b[1], 2 * b[2], b[3], 2 * b[4]]

    for L, I in (('enc0', 2), ('dec0', 1)):
        F = I + HID
        eA = basis(L + '_Wru', I)
        eC = basis(L + '_Wc', I)
        wa_id = np.zeros((P, P), f32)
        wa_id[0:HID] = eA[0][I:F]
        if L == 'enc0':
            wa_id[HID:HID + I] = eA[0][0:I]
        common[L + '_wa_id'] = wa_id.astype(f16)
        wa_hs = np.zeros((P, 4, P), f32)
        for m in range(4):
            wa_hs[HID:P, m] = eA[m + 1][I:F]
        common[L + '_wa_hs'] = wa_hs.astype(f16)
        waq = np.zeros((4 * I, 4, CHAINS, P), f32)
        wcq = np.zeros((4 * I, 4, CHAINS, HID), f32)
        for m in range(4):
            for c in range(CHAINS):
                waq[c * I:(c + 1) * I, m, c] = eA[m + 1][0:I]
                wcq[c * I:(c + 1) * I, m, c] = eC[m + 1][0:I]
        common[L + '_waq'] = waq.astype(f16)
        common[L + '_wcq'] = wcq.astype(f16)
        common[L + '_wc0h'] = np.ascontiguousarray(eC[0][I:F]).astype(f16)
        for hh in range(2):
            wcr = np.zeros((P, 4, HID), f32)
            for m in range(4):
                wcr[hh * HID:(hh + 1) * HID, m] = eC[m + 1][I:F]
            common[L + f'_wcr{hh}'] = wcr.astype(f16)
        bru = np.asarray(inputs[L + '_bru'], f32)
        common[L + '_bru'] = np.stack([bru[0:HID], bru[HID:P]], axis=1).copy()
        common[L + '_bc'] = np.asarray(inputs[L + '_bc'], f32).reshape(HID, 1).copy()
        if L == 'enc0':
            wc0x = np.zeros((P, HID), f32)
            wc0x[HID:HID + I] = eC[0][0:I]
            common['enc0_wc0x'] = wc0x.astype(f16)
        else:
            common['dec0_wa0y'] = np.ascontiguousarray(eA[0][0:1]).astype(f16)
            common['dec0_wc0y'] = np.ascontiguousarray(eC[0][0:1]).astype(f16)

    for L, I in L1LAYERS:
        F = I + HID
        eA = basis(L + '_Wru', I)
        eC = basis(L + '_Wc', I)
        wa = np.zeros((P, 5, P), f32)
        for i in range(5):
            wa[0:HID, i] = eA[i][I:F]
            if i > 0:
                wa[HID:P, i] = eA[i][0:I]
        common[L + '_wa'] = wa.astype(f16)
        common[L + '_wa0x'] = np.ascontiguousarray(eA[0][0:I]).astype(f16)
        common[L + '_wc0h'] = np.ascontiguousarray(eC[0][I:F]).astype(f16)
        common[L + '_wc0x'] = np.ascontiguousarray(eC[0][0:I]).astype(f16)
        wcx = np.zeros((P, 4, HID), f32)
        for m in range(4):
            wcx[HID:P, m] = eC[m + 1][0:I]
        common[L + '_wcx'] = wcx.astype(f16)
        for hh in range(2):
            wcr = np.zeros((P, 4, HID), f32)
            for m in range(4):
                wcr[hh * HID:(hh + 1) * HID, m] = eC[m + 1][I:F]
            common[L + f'_wcr{hh}'] = wcr.astype(f16)
        bru = np.asarray(inputs[L + '_bru'], f32)
        common[L + '_bru'] = np.stack([bru[0:HID], bru[HID:P]], axis=1).copy()
        common[L + '_bc'] = np.asarray(inputs[L + '_bc'], f32).reshape(HID, 1).copy()

    common['zeros'] = np.zeros((P, N), f16)
    fw = np.asarray(inputs['fcn_W'], f32).reshape(HID, 1)
    common['fcnw1'] = fw.astype(f16).copy()
    fb = np.asarray(inputs['fcn_b'], f32).reshape(-1)[0]
    common['fcnb1'] = np.full((1, 1), fb, f32)
    common['fcnb128'] = np.full((P, 1), fb, f32)

    T = _cache['T']
    in_maps = []
    for core in range(N_CORES):
        m = dict(common)
        xT = np.zeros((CHAINS, T, 2, N), f16)
        xq = np.zeros((P, T, KT, 8), f16)
        for c in range(CHAINS):
            b = core * CHAINS + c
            for t in range(T):
                xt = source[b, t]            # [N, 2]
                xT[c, t] = xt.T.astype(f16)
                xq[:, t, :, 2 * c:2 * c + 2] = \
                    xt.reshape(KT, P, 2).transpose(1, 0, 2).astype(f16)
        m['xT'] = xT
        m['xq'] = xq
        in_maps.append(m)
    return in_maps


def kernel(**inputs):
    from concourse import bass_utils

    T = int(os.environ.get('DCRNN_T', T_FULL))
    HOR = int(os.environ.get('DCRNN_HOR', HOR_FULL))
    key = (T, HOR)
    if _cache.get('key') != key:
        _cache['nc'] = _build(T, HOR)
        _cache['key'] = key
    _cache['T'] = T
    _cache['HOR'] = HOR

    in_maps = _prep_host(inputs)
    res = bass_utils.run_bass_kernel_spmd(
        _cache['nc'], in_maps, core_ids=list(range(N_CORES)),
        trace=bool(int(os.environ.get('DCRNN_TRACE', '0'))))
    _cache['last_res'] = res

    HORr = _cache['HOR']
    out = np.zeros((HORr, B, N), np.float32)
    for core in range(N_CORES):
        r = res.results[core]['out']         # [HOR, CHAINS, N] fp16
        for c in range(CHAINS):
            out[:, core * CHAINS + c, :] = r[:, c, :].astype(np.float32)
    return out


# revision 17
# speedup vs baseline: 3.0636x; 1.1231x over previous
"""DCRNN (nn_DCRNNModel) forward pass on 8 Trainium2 NeuronCores.

Data-parallel over batch (B=32 -> 4 chains/core). All weights plus the four
diffusion operators S0, S0^2, S1, S1^2 (S^2 precomputed on host) live in
SBUF in fp16 for the whole kernel. All PE matmuls run in fp16 (1 cycle/row
at any moving size); PSUM accumulates fp32.

Diffusion reuse: layer 1 diffuses zn1=[h1|h0'] per chain; its V tiles hold
S^k h0' in rows 64:128, which layer 0 of the NEXT step reuses as its h-part
diffusion (via row-shifted weights), so layer 0 only diffuses the tiny
x-quad pack (all 4 chains' inputs in one 8-feature pack, 4 sets/step total).
Gate B diffuses only r*h, pair-packed two chains per 128-wide pack; its
x-terms reuse the V tiles through zero-padded K=128 weights so every matmul
operand sits at partition base 0 (HW requires equal base partitions and a
stable PE tile row inside an accumulation group).
Loops are ordered so consecutive matmuls share the same stationary operand
(one weight load per kt instead of per (kt,slice)).
"""
import sys
import os
import time as _time
import numpy as np

sys.path.insert(0, '/opt/trn_rl_repo')

N = 1024
B = 32
T_FULL = 12
HOR_FULL = 12
HID = 64
N_CORES = 8
CHAINS = 4
KT = 8
P = 128

_cache = {}

L1LAYERS = (('enc1', 64), ('dec1', 64))


def _build(T, HOR):
    import concourse.bacc as bacc
    import concourse.tile as tile
    from concourse import mybir

    F32 = mybir.dt.float32
    F16 = mybir.dt.float16
    AF = mybir.ActivationFunctionType

    _t0 = _time.time()
    nc = bacc.Bacc('TRN2', target_bir_lowering=False, debug=False,
                   num_devices=N_CORES)

    # ---- DRAM declarations ----
    d = {}
    for m in range(4):
        d[f's_t{m}'] = nc.dram_tensor(f's_t{m}', [P, KT, N], F16,
                                      kind='ExternalInput').ap()
    d['ident'] = nc.dram_tensor('ident', [P, P], F16, kind='ExternalInput').ap()
    d['xT'] = nc.dram_tensor('xT', [CHAINS, T, 2, N], F16,
                             kind='ExternalInput').ap()
    d['xq'] = nc.dram_tensor('xq', [P, T, KT, 8], F16, kind='ExternalInput').ap()
    d['zeros'] = nc.dram_tensor('zeros', [P, N], F16, kind='ExternalInput').ap()
    # layer-0 weights (enc0 I=2, dec0 I=1)
    for L, I in (('enc0', 2), ('dec0', 1)):
        d[L + '_wa_id'] = nc.dram_tensor(L + '_wa_id', [P, P], F16,
                                         kind='ExternalInput').ap()
        d[L + '_wa_hs'] = nc.dram_tensor(L + '_wa_hs', [P, 4, P], F16,
                                         kind='ExternalInput').ap()
        d[L + '_waq'] = nc.dram_tensor(L + '_waq', [4 * I, 4, CHAINS, P], F16,
                                       kind='ExternalInput').ap()
        d[L + '_wcq'] = nc.dram_tensor(L + '_wcq', [4 * I, 4, 2, P], F16,
                                       kind='ExternalInput').ap()
        d[L + '_wc0h'] = nc.dram_tensor(L + '_wc0h', [P, P], F16,
                                        kind='ExternalInput').ap()
        d[L + '_wcr'] = nc.dram_tensor(L + '_wcr', [P, 4, P], F16,
                                       kind='ExternalInput').ap()
        d[L + '_bru'] = nc.dram_tensor(L + '_bru', [HID, 2], F32,
                                       kind='ExternalInput').ap()
        d[L + '_bc'] = nc.dram_tensor(L + '_bc', [HID, 1], F32,
                                      kind='ExternalInput').ap()
    d['enc0_wc0x'] = nc.dram_tensor('enc0_wc0x', [P, 2, P], F16,
                                    kind='ExternalInput').ap()
    d['dec0_wa0y'] = nc.dram_tensor('dec0_wa0y', [1, P], F16,
                                    kind='ExternalInput').ap()
    d['dec0_wc0y'] = nc.dram_tensor('dec0_wc0y', [1, 2, P], F16,
                                    kind='ExternalInput').ap()
    # layer-1 weights (I=64)
    for L, I in L1LAYERS:
        d[L + '_wa'] = nc.dram_tensor(L + '_wa', [P, 5, P], F16,
                                      kind='ExternalInput').ap()
        d[L + '_wa0x'] = nc.dram_tensor(L + '_wa0x', [HID, P], F16,
                                        kind='ExternalInput').ap()
        d[L + '_wc0h'] = nc.dram_tensor(L + '_wc0h', [P, P], F16,
                                        kind='ExternalInput').ap()
        d[L + '_wc0x'] = nc.dram_tensor(L + '_wc0x', [HID, 2, P], F16,
                                        kind='ExternalInput').ap()
        d[L + '_wcx'] = nc.dram_tensor(L + '_wcx', [P, 4, 2, P], F16,
                                       kind='ExternalInput').ap()
        d[L + '_wcr'] = nc.dram_tensor(L + '_wcr', [P, 4, P], F16,
                                       kind='ExternalInput').ap()
        d[L + '_bru'] = nc.dram_tensor(L + '_bru', [HID, 2], F32,
                                       kind='ExternalInput').ap()
        d[L + '_bc'] = nc.dram_tensor(L + '_bc', [HID, 1], F32,
                                      kind='ExternalInput').ap()
    d['fcnw1'] = nc.dram_tensor('fcnw1', [HID, 1], F16, kind='ExternalInput').ap()
    d['fcnb1'] = nc.dram_tensor('fcnb1', [1, 1], F32, kind='ExternalInput').ap()
    d['fcnb128'] = nc.dram_tensor('fcnb128', [P, 1], F32, kind='ExternalInput').ap()
    d_out = nc.dram_tensor('out', [HOR, CHAINS, N], F16, kind='ExternalOutput').ap()

    with tile.TileContext(nc) as tc:
        with tc.tile_pool(name='const', bufs=1) as const, \
             tc.tile_pool(name='state', bufs=1) as state, \
             tc.tile_pool(name='vl', bufs=16) as vlp, \
             tc.tile_pool(name='vp', bufs=4) as vpp, \
             tc.tile_pool(name='vq', bufs=4) as vqp, \
             tc.tile_pool(name='rr', bufs=2) as rrp, \
             tc.tile_pool(name='uu', bufs=5) as uup, \
             tc.tile_pool(name='rh', bufs=2) as rhp, \
             tc.tile_pool(name='ct', bufs=4) as ctp, \
             tc.tile_pool(name='tg', bufs=1) as tgp, \
             tc.tile_pool(name='pd', bufs=2, space='PSUM') as pdp, \
             tc.tile_pool(name='prj', bufs=2, space='PSUM') as prj:

            # ---- load constants ----
            cst = {}
            for name, dd in d.items():
                if name in ('xT', 'zeros'):
                    continue
                t = const.tile(list(dd.shape), dd.dtype, tag=name,
                               name='cst_' + name)
                nc.sync.dma_start(t[:], dd[:])
                cst[name] = t
            s_t = [cst[f's_t{m}'] for m in range(4)]
            ident = cst['ident']
            d_zeros = d['zeros']
            zn_zeros = d_zeros.rearrange('p (kt f) -> p kt f', kt=KT)

            # ---- persistent state ----
            zT0, zT1, zn1, ytmp = [], [], [], []
            for c in range(CHAINS):
                zT0.append(state.tile([P, N], F16, tag=f'zT0_{c}', name=f'zT0_{c}'))
                zT1.append(state.tile([HID, N], F16, tag=f'zT1_{c}', name=f'zT1_{c}'))
                zn1.append(state.tile([P, KT, P], F16, tag=f'zn1_{c}', name=f'zn1_{c}'))
                ytmp.append(state.tile([1, N], F16, tag=f'ytmp_{c}', name=f'ytmp_{c}'))
                nc.sync.dma_start(zT0[c][:], d_zeros[:])
                nc.sync.dma_start(zT1[c][:], d_zeros[0:HID, :])
                nc.sync.dma_start(zn1[c][:], zn_zeros[:])
            zr = [state.tile([P, KT, P], F16, tag=f'zr_{pr}', name=f'zr_{pr}')
                  for pr in range(2)]
            yq = state.tile([P, KT, CHAINS], F16, tag='yq', name='yq')

            vl1 = {}     # (c, m) -> V tile of layer-1 diffusion (prev step)

            copy_rr = [0]

            def copy_psum(dst, src):
                copy_rr[0] += 1
                if copy_rr[0] % 2:
                    nc.scalar.activation(dst, src, AF.Copy)
                else:
                    nc.vector.tensor_copy(dst, src)

            def diffuse(lhs_fn, m, pool, tag, nfeat=P):
                """V = (S_m z)^T feature-major [nfeat, N] fp16 SBUF tile.
                lhs_fn(kt) -> lhsT AP [128, nfeat]."""
                pd = pdp.tile([P, N], F32, tag='pd')
                stt = s_t[m]
                for kt in range(KT):
                    for ms in range(2):
                        sl = slice(ms * 512, (ms + 1) * 512)
                        nc.tensor.matmul(pd[0:nfeat, sl], lhs_fn(kt),
                                         stt[:, kt, sl],
                                         start=(kt == 0), stop=(kt == KT - 1))
                v = pool.tile([nfeat, N], F16, tag=tag)
                copy_psum(v[:], pd[0:nfeat, :])
                return v

            def proj(ptile, psl, mms):
                """Accumulate terms into ptile[psl, :]; term-outer order so
                both 512-slices of one term share the weight load."""
                nmm = len(mms)
                for j, (lhs, rhs) in enumerate(mms):
                    for ms in range(2):
                        sl = slice(ms * 512, (ms + 1) * 512)
                        nc.tensor.matmul(ptile[psl, sl], lhs, rhs[:, sl],
                                         start=(j == 0), stop=(j == nmm - 1))

            def sigmoid_ru(L, pa):
                rr = rrp.tile([HID, N], F16, tag='rr')
                uu = uup.tile([HID, N], F16, tag='uu')
                nc.scalar.activation(rr[:], pa[0:HID, :], AF.Sigmoid,
                                     bias=cst[L + '_bru'][:, 0:1])
                nc.scalar.activation(uu[:], pa[HID:P, :], AF.Sigmoid,
                                     bias=cst[L + '_bru'][:, 1:2])
                return rr, uu

            def gru_and_transpose(zh, ct_c, uu, dst_znc, dst_off):
                """h' = c + u*(h-c); transpose h' into dst_znc[:, :, off]."""
                t1 = tgp.tile([HID, N], F16, tag='tg')
                nc.vector.tensor_sub(t1[:], zh[0:HID, :], ct_c[:])
                nc.vector.tensor_mul(t1[:], t1[:], uu[:])
                nc.vector.tensor_add(zh[0:HID, :], ct_c[:], t1[:])
                pth = pdp.tile([P, KT, HID], F16, tag='pd')
                for kt in range(KT):
                    nc.tensor.transpose(pth[:, kt, :],
                                        zh[0:HID, kt * P:(kt + 1) * P],
                                        ident[0:HID, 0:HID])
                copy_psum(dst_znc[:, :, dst_off:dst_off + HID], pth[:])

            def gA_l0(L, c0, vq, skip_h):
                """Layer-0 gate A for chains (c0,c0+1): proj+sigmoid+r*h.
                Returns (rh_pair, ru_u)."""
                ru_u = {}
                rhpr = rhp.tile([P, N], F16, tag='rh')
                for c in (c0, c0 + 1):
                    pa = prj.tile([P, N], F32, tag='prj')
                    mms = [(cst[L + '_wa_id'][:], zT0[c])]
                    if L == 'dec0':
                        mms.append((cst['dec0_wa0y'][:], ytmp[c]))
                    if not skip_h:
                        for m in range(4):
                            mms.append((cst[L + '_wa_hs'][:, m, :], vl1[(c, m)]))
                    for m in range(4):
                        mms.append((cst[L + '_waq'][:, m, c, :], vq[m]))
                    proj(pa, slice(0, P), mms)
                    rr, ru_u[c] = sigmoid_ru(L, pa)
                    off = (c - c0) * HID
                    nc.vector.tensor_mul(rhpr[off:off + HID, :], rr[:],
                                         zT0[c][0:HID, :])
                return rhpr, ru_u

            def gA_l1(L, c0):
                """Layer-1 gate A: diffuse zn1, proj+sigmoid+r*h."""
                wa = cst[L + '_wa']
                for c in (c0, c0 + 1):
                    for m in range(4):
                        vl1[(c, m)] = diffuse(
                            lambda kt, c=c: zn1[c][:, kt, :], m, vlp, 'vl')
                ru_u = {}
                rhpr = rhp.tile([P, N], F16, tag='rh')
                for c in (c0, c0 + 1):
                    pa = prj.tile([P, N], F32, tag='prj')
                    mms = [(wa[0:HID, 0, :], zT1[c][0:HID, :]),
                           (cst[L + '_wa0x'][:], zT0[c][0:HID, :])]
                    for m in range(4):
                        mms.append((wa[:, m + 1, :], vl1[(c, m)]))
                    proj(pa, slice(0, P), mms)
                    rr, ru_u[c] = sigmoid_ru(L, pa)
                    off = (c - c0) * HID
                    nc.vector.tensor_mul(rhpr[off:off + HID, :], rr[:],
                                         zT1[c][0:HID, :])
                return rhpr, ru_u

            def gB_diff(pair, rhpr):
                """Transpose rh pair-pack to node-major, diffuse it."""
                ptr = pdp.tile([P, KT, P], F16, tag='pd')
                for kt in range(KT):
                    nc.tensor.transpose(ptr[:, kt, :],
                                        rhpr[:, kt * P:(kt + 1) * P], ident[:])
                nc.vector.tensor_copy(zr[pair][:], ptr[:])
                return [diffuse(lambda kt, pr=pair: zr[pr][:, kt, :],
                                m, vpp, 'vp') for m in range(4)]

            def gB_proj(L, c0, rhpr, vq, vps):
                """Pair-packed gate-B projection + per-chain tanh."""
                pair = c0 // 2
                pb = prj.tile([P, N], F32, tag='prj')
                mms = [(cst[L + '_wc0h'][:], rhpr)]
                for c in (c0, c0 + 1):
                    hh = c - c0
                    if L == 'enc0':
                        mms.append((cst['enc0_wc0x'][:, hh, :], zT0[c]))
                    elif L == 'dec0':
                        mms.append((cst['dec0_wc0y'][:, hh, :], ytmp[c]))
                    else:
                        mms.append((cst[L + '_wc0x'][:, hh, :], zT0[c][0:HID, :]))
                for m in range(4):
                    if vq is not None:
                        mms.append((cst[L + '_wcq'][:, m, pair, :], vq[m]))
                    else:
                        for c in (c0, c0 + 1):
                            hh = c - c0
                            mms.append((cst[L + '_wcx'][:, m, hh, :], vl1[(c, m)]))
                    mms.append((cst[L + '_wcr'][:, m, :], vps[m]))
                proj(pb, slice(0, P), mms)
                ct = {}
                for c in (c0, c0 + 1):
                    hh = c - c0
                    ct_c = ctp.tile([HID, N], F16, tag='ct')
                    nc.scalar.activation(ct_c[:], pb[hh * HID:(hh + 1) * HID, :],
                                         AF.Tanh, bias=cst[L + '_bc'][:])
                    ct[c] = ct_c
                return ct

            def layer0(L, vq, skip_h):
                A = [gA_l0(L, 2 * pr, vq, skip_h) for pr in range(2)]
                C = []
                for pr in range(2):
                    vps = gB_diff(pr, A[pr][0])
                    C.append(gB_proj(L, 2 * pr, A[pr][0], vq, vps))
                for pr in range(2):
                    for c in (2 * pr, 2 * pr + 1):
                        gru_and_transpose(zT0[c], C[pr][c], A[pr][1][c],
                                          zn1[c], HID)

            def layer1(L, y_t=None):
                A = [gA_l1(L, 2 * pr) for pr in range(2)]
                C = []
                for pr in range(2):
                    vps = gB_diff(pr, A[pr][0])
                    C.append(gB_proj(L, 2 * pr, A[pr][0], None, vps))
                for pr in range(2):
                    for c in (2 * pr, 2 * pr + 1):
                        gru_and_transpose(zT1[c], C[pr][c], A[pr][1][c],
                                          zn1[c], 0)
                    if y_t is not None:
                        y_stage(y_t, pr)

            def y_stage(t, pr):
                pq = pdp.tile([P, KT, 2], F32, tag='pd')
                for c in (2 * pr, 2 * pr + 1):
                    py = pdp.tile([1, N], F32, tag='pd')
                    for ms in range(2):
                        sl = slice(ms * 512, (ms + 1) * 512)
                        nc.tensor.matmul(py[:, sl], cst['fcnw1'][:],
                                         zT1[c][0:HID, sl],
                                         start=True, stop=True)
                    nc.scalar.activation(ytmp[c][:], py[:], AF.Identity,
                                         bias=cst['fcnb1'][:])
                    nc.sync.dma_start(d_out[t, c:c + 1, :], ytmp[c][:])
                    if t < HOR - 1:
                        for kt in range(KT):
                            nc.tensor.matmul(
                                pq[:, kt, c - 2 * pr:c - 2 * pr + 1],
                                zT1[c][0:HID, kt * P:(kt + 1) * P],
                                cst['fcnw1'][:], start=True, stop=True)
                if t < HOR - 1:
                    nc.scalar.activation(yq[:, :, 2 * pr:2 * pr + 2],
                                         pq[:], AF.Identity,
                                         bias=cst['fcnb128'][:])

            # ================= encoder =================
            xq_t = cst['xq']

            def enc_vq(t):
                return [diffuse(lambda kt, t=t: xq_t[:, t, kt, :], m,
                                vqp, 'vq', nfeat=8) for m in range(4)]

            vq = enc_vq(0)
            for t in range(T):
                for c in range(CHAINS):
                    nc.sync.dma_start(zT0[c][HID:HID + 2, :], d['xT'][c, t])
                layer0('enc0', vq, skip_h=(t == 0))
                if t + 1 < T:
                    vq_next = enc_vq(t + 1)
                layer1('enc1')
                if t + 1 < T:
                    vq = vq_next

            # ================= decoder =================
            for c in range(CHAINS):
                nc.sync.dma_start(zT0[c][HID:HID + 2, :], d_zeros[0:2, :])
                nc.sync.dma_start(ytmp[c][:], d_zeros[0:1, :])
            nc.sync.dma_start(
                yq[:], d_zeros[:, 0:KT * CHAINS].rearrange(
                    'p (kt f) -> p kt f', kt=KT))
            for t in range(HOR):
                vq = [diffuse(lambda kt: yq[:, kt, :], m, vqp, 'vq',
                              nfeat=CHAINS) for m in range(4)]
                layer0('dec0', vq, skip_h=False)
                layer1('dec1', y_t=t)

    print(f'[build] emission+schedule: {_time.time() - _t0:.1f}s', flush=True)
    _t1 = _time.time()
    nc.compile()
    print(f'[build] bacc compile: {_time.time() - _t1:.1f}s', flush=True)
    return nc


def _prep_host(inputs):
    """Host-side preprocessing -> per-core in_maps (fp16)."""
    f32 = np.float32
    f16 = np.float16
    adj = np.asarray(inputs['adj'], f32)
    source = np.asarray(inputs['source'], f32)       # [B, T, N, 2]

    def rw(a):
        return (a / np.maximum(a.sum(1, keepdims=True), np.float32(1e-8))).astype(f32)

    s0 = rw(adj)
    s1 = rw(adj.T)
    mats = [s0, s0 @ s0, s1, s1 @ s1]

    def tile_nm(a):
        return np.ascontiguousarray(
            a.reshape(KT, P, N).transpose(1, 0, 2)).astype(f16)

    common = {f's_t{m}': tile_nm(mats[m]) for m in range(4)}
    common['ident'] = np.eye(P, dtype=f16)

    def basis(Wname, I):
        F = I + HID
        W = np.asarray(inputs[Wname], f32)
        b = [W[i * F:(i + 1) * F] for i in range(5)]
        return [b[0] - b[2] - b[4], b[1], 2 * b[2], b[3], 2 * b[4]]

    for L, I in (('enc0', 2), ('dec0', 1)):
        F = I + HID
        eA = basis(L + '_Wru', I)
        eC = basis(L + '_Wc', I)
        wa_id = np.zeros((P, P), f32)
        wa_id[0:HID] = eA[0][I:F]
        if L == 'enc0':
            wa_id[HID:HID + I] = eA[0][0:I]
        common[L + '_wa_id'] = wa_id.astype(f16)
        wa_hs = np.zeros((P, 4, P), f32)
        for m in range(4):
            wa_hs[HID:P, m] = eA[m + 1][I:F]
        common[L + '_wa_hs'] = wa_hs.astype(f16)
        waq = np.zeros((4 * I, 4, CHAINS, P), f32)
        wcq = np.zeros((4 * I, 4, 2, P), f32)
        for m in range(4):
            for c in range(CHAINS):
                waq[c * I:(c + 1) * I, m, c] = eA[m + 1][0:I]
                pr, hh = c // 2, c % 2
                wcq[c * I:(c + 1) * I, m, pr,
                    hh * HID:(hh + 1) * HID] = eC[m + 1][0:I]
        common[L + '_waq'] = waq.astype(f16)
        common[L + '_wcq'] = wcq.astype(f16)
        wc0h = np.zeros((P, P), f32)
        wcr = np.zeros((P, 4, P), f32)
        for hh in range(2):
            cs = slice(hh * HID, (hh + 1) * HID)
            wc0h[cs, cs] = eC[0][I:F]
            for m in range(4):
                wcr[cs, m, cs] = eC[m + 1][I:F]
        common[L + '_wc0h'] = wc0h.astype(f16)
        common[L + '_wcr'] = wcr.astype(f16)
        bru = np.asarray(inputs[L + '_bru'], f32)
        common[L + '_bru'] = np.stack([bru[0:HID], bru[HID:P]], axis=1).copy()
        common[L + '_bc'] = np.asarray(inputs[L + '_bc'], f32).reshape(HID, 1).copy()
        if L == 'enc0':
            wc0x = np.zeros((P, 2, P), f32)
            for hh in range(2):
                wc0x[HID:HID + I, hh, hh * HID:(hh + 1) * HID] = eC[0][0:I]
            common['enc0_wc0x'] = wc0x.astype(f16)
        else:
            common['dec0_wa0y'] = np.ascontiguousarray(eA[0][0:1]).astype(f16)
            wc0y = np.zeros((1, 2, P), f32)
            for hh in range(2):
                wc0y[0, hh, hh * HID:(hh + 1) * HID] = eC[0][0:1]
            common['dec0_wc0y'] = wc0y.astype(f16)

    for L, I in L1LAYERS:
        F = I + HID
        eA = basis(L + '_Wru', I)
        eC = basis(L + '_Wc', I)
        wa = np.zeros((P, 5, P), f32)
        for i in range(5):
            wa[0:HID, i] = eA[i][I:F]
            if i > 0:
                wa[HID:P, i] = eA[i][0:I]
        common[L + '_wa'] = wa.astype(f16)
        common[L + '_wa0x'] = np.ascontiguousarray(eA[0][0:I]).astype(f16)
        wc0h = np.zeros((P, P), f32)
        wcr = np.zeros((P, 4, P), f32)
        for hh in range(2):
            cs = slice(hh * HID, (hh + 1) * HID)
            wc0h[cs, cs] = eC[0][I:F]
            for m in range(4):
                wcr[cs, m, cs] = eC[m + 1][I:F]
        common[L + '_wc0h'] = wc0h.astype(f16)
        common[L + '_wcr'] = wcr.astype(f16)
        wc0x = np.zeros((HID, 2, P), f32)
        wcx = np.zeros((P, 4, 2, P), f32)
        for hh in range(2):
            cs = slice(hh * HID, (hh + 1) * HID)
            wc0x[:, hh, cs] = eC[0][0:I]
            for m in range(4):
                wcx[HID:P, m, hh, cs] = eC[m + 1][0:I]
        common[L + '_wc0x'] = wc0x.astype(f16)
        common[L + '_wcx'] = wcx.astype(f16)
        bru = np.asarray(inputs[L + '_bru'], f32)
        common[L + '_bru'] = np.stack([bru[0:HID], bru[HID:P]], axis=1).copy()
        common[L + '_bc'] = np.asarray(inputs[L + '_bc'], f32).reshape(HID, 1).copy()

    common['zeros'] = np.zeros((P, N), f16)
    fw = np.asarray(inputs['fcn_W'], f32).reshape(HID, 1)
    common['fcnw1'] = fw.astype(f16).copy()
    fb = np.asarray(inputs['fcn_b'], f32).reshape(-1)[0]
    common['fcnb1'] = np.full((1, 1), fb, f32)
    common['fcnb128'] = np.full((P, 1), fb, f32)

    T = _cache['T']
    in_maps = []
    for core in range(N_CORES):
        m = dict(common)
        xT = np.zeros((CHAINS, T, 2, N), f16)
        xq = np.zeros((P, T, KT, 8), f16)
        for c in range(CHAINS):
            b = core * CHAINS + c
            for t in range(T):
                xt = source[b, t]            # [N, 2]
                xT[c, t] = xt.T.astype(f16)
                xq[:, t, :, 2 * c:2 * c + 2] = \
                    xt.reshape(KT, P, 2).transpose(1, 0, 2).astype(f16)
        m['xT'] = xT
        m['xq'] = xq
        in_maps.append(m)
    return in_maps


def kernel(**inputs):
    from concourse import bass_utils

    T = int(os.environ.get('DCRNN_T', T_FULL))
    HOR = int(os.environ.get('DCRNN_HOR', HOR_FULL))
    key = (T, HOR)
    if _cache.get('key') != key:
        _cache['nc'] = _build(T, HOR)
        _cache['key'] = key
    _cache['T'] = T
    _cache['HOR'] = HOR

    in_maps = _prep_host(inputs)
    res = bass_utils.run_bass_kernel_spmd(
        _cache['nc'], in_maps, core_ids=list(range(N_CORES)),
        trace=bool(int(os.environ.get('DCRNN_TRACE', '0'))))
    _cache['last_res'] = res

    HORr = _cache['HOR']
    out = np.zeros((HORr, B, N), np.float32)
    for core in range(N_CORES):
        r = res.results[core]['out']         # [HOR, CHAINS, N] fp16
        for c in range(CHAINS):
            out[:, core * CHAINS + c, :] = r[:, c, :].astype(np.float32)
    return out
